# revision 17
# baseline (speedup 1.0000x reference)
"""NlmCNN (weight-predicting CNN + per-pixel 13x13 weighted sum) on 8 trn2 cores.

Sharding: data-parallel over batch (8 images -> 8 cores), weights replicated.

Per-core layout trick: output y is the conv stack's result center-cropped by
6 pixels, and the receptive field of the three 3x3 convs only reaches 3 px
out, so SAME-padding never materializes: every layer is computed VALID-style
on an unpadded 256-stride flat layout. Column-wrap junk from flat shifted
reads stays confined to the outer <=3 columns of each layer, which are
discarded by the crop.

Pipeline per strip of S output rows (strips software-pipelined: conv1 of
strip i+1 is emitted during strip i's conv3 phase):
  conv1: per-2-chunk im2col [9, 1024] via one 3-dim DMA -> K=9 matmul
  conv2: taps (du,0)+(du,1) fused into K=128 matmuls against an SBUF tile
         whose upper 64 partitions hold h shifted by +1 (built by two
         SBUF->SBUF SWDGE DMAs per strip); taps (du,2) are K=64 singles
  conv3: out channels split [0:128] (M=128) / [128:169] (M=41). Taps
         (du,0)+(du,1) pair on the h2A=[h2; h2<<1] tile as in conv2; taps
         (0,2)+(1,2) pair on a second h2B=[h2; h2<<W] tile (two more dup
         DMAs per strip); only tap (2,2) is a K=64 single. 10 matmuls per
         512-px chunk instead of 12. Chunk parity alternates
         [singles | stair | pairs] / [pairs | stair | singles] so the PE
         sees one K-row-size transition per chunk.
  einsum: patch matrix xs[t, s] = x[pos + shift(t)] split [128 | 41] taps,
         gathered bf16 by one contiguous DMA per tap-row u (row u=9 is
         split 11/2 across the two tiles); DVE scalar_tensor_tensor
         computes t2[0:128] = (conv3_lo + b3_lo) * xs_lo from PSUM and
         t_hi = (conv3_hi + b3_hi) * xs_hi; gpsimd folds t_hi into
         t2[0:41] so the partition reduction is ONE K=128 "staircase
         ones" matmul per 2-row chunk (writes row jj of a persistent PSUM
         tile); one copy + one DMA store the whole image.

All matmul operands are float32r (fp32 streamed at bf16 rate, ~1e-4
relerr). Measured on HW: bf16 MOVING operands stream at HALF the fp32r
rate on this PE, so fp32r is the fast path; mixing dtypes is rejected by
the ISA, and tile_position array-packing hangs the PE with fp32r. xs is
gathered bf16 (it only feeds the DVE, not the PE).
"""

import numpy as np

import concourse.bacc as bacc
import concourse.bass as bass
import concourse.mybir as mybir
import concourse.tile as tile
from concourse.bass_utils import run_bass_kernel_spmd

F32 = mybir.dt.float32
F32R = mybir.dt.float32r
BF16 = mybir.dt.bfloat16
AF = mybir.ActivationFunctionType
ALU = mybir.AluOpType

H = 256
W = 256
K = 13
HO = H - K + 1  # 244
CH = 64
C3 = K * K  # 169
CLO = 128   # conv3 out-channel group sizes
CHI = C3 - 128  # 41
S_STRIP = 14
NC_ = 512  # chunk positions (2 image rows)
import os
PIPE = os.environ.get("K_PIPE", "1") == "1"      # cross-strip sw pipelining
XS_GP = os.environ.get("K_XS_GP", "1") == "1"    # xs DMAs on gpsimd queue
DUP_GP = os.environ.get("K_DUP_GP", "1") == "1"  # dup DMAs on gpsimd queue


def _ap(t, off, dims):
    return bass.AP(t, off, [list(d) for d in dims])


def _mm(nc, out, lhsT, rhs, start, stop):
    nc.tensor.matmul(out, lhsT, rhs, start=start, stop=stop)


def build_nc():
    nc = bacc.Bacc("TRN2", target_bir_lowering=False, debug=False)

    x = nc.dram_tensor("x", [1, 1, H, W], F32, kind="ExternalInput")
    w1 = nc.dram_tensor("W1", [CH, 1, 3, 3], F32, kind="ExternalInput")
    b1 = nc.dram_tensor("b1", [CH], F32, kind="ExternalInput")
    w2 = nc.dram_tensor("W2", [CH, CH, 3, 3], F32, kind="ExternalInput")
    b2 = nc.dram_tensor("b2", [CH], F32, kind="ExternalInput")
    w3 = nc.dram_tensor("W3", [C3, CH, 3, 3], F32, kind="ExternalInput")
    b3 = nc.dram_tensor("b3", [C3], F32, kind="ExternalInput")
    y = nc.dram_tensor("y", [1, 1, HO, HO], F32, kind="ExternalOutput")
    xr = nc.dram_tensor("x_r", [H * W], F32R)
    xb = nc.dram_tensor("x_b", [H * W], BF16)

    with tile.TileContext(nc) as tc:
        Body(nc, tc, x, w1, b1, w2, b2, w3, b3, y, xr, xb).build()

    nc.compile()
    return nc


class Body:
    def __init__(self, nc, tc, x, w1, b1, w2, b2, w3, b3, y, xr, xb):
        self.nc, self.tc = nc, tc
        self.x, self.w1, self.b1, self.w2, self.b2 = x, w1, b1, w2, b2
        self.w3, self.b3, self.y, self.xr, self.xb = w3, b3, y, xr, xb

    def build(self):
        nc, tc = self.nc, self.tc
        with (
            tc.tile_pool(name="consts", bufs=1) as consts,
            tc.tile_pool(name="t2p", bufs=5) as p_t2,
            tc.tile_pool(name="thp", bufs=5) as p_th,
            tc.tile_pool(name="imc", bufs=5) as p_imc,
            tc.tile_pool(name="h1p", bufs=2) as p_h1,
            tc.tile_pool(name="h2p", bufs=2) as p_h2,
            tc.tile_pool(name="h2bp", bufs=2) as p_h2b,
            tc.tile_pool(name="xsl", bufs=2) as p_xsl,
            tc.tile_pool(name="xsh", bufs=2) as p_xsh,
            tc.tile_pool(name="yout", bufs=1) as p_y,
            tc.tile_pool(name="ps12", bufs=2, space="PSUM") as ps12,
            tc.tile_pool(name="ps3", bufs=2, space="PSUM") as ps3,
            tc.tile_pool(name="psy", bufs=1, space="PSUM") as psy,
        ):
            self.consts = consts
            self.p_t2, self.p_th, self.p_imc = p_t2, p_th, p_imc
            self.p_h1, self.p_h2, self.p_h2b = p_h1, p_h2, p_h2b
            self.p_xsl, self.p_xsh = p_xsl, p_xsh
            self.p_y, self.ps12, self.ps3, self.psy = p_y, ps12, ps3, psy
            self._build_consts()
            self._build_strips()

    def _build_consts(self):
        nc, tc, consts = self.nc, self.tc, self.consts
        stage = tc.alloc_tile_pool(name="stage", bufs=1)
        # weight-prep transposes borrow ps3's "ps3lo" slot (same max tile
        # size, consts-time only) so no dedicated PSUM bank is needed
        pwtr = self.ps3

        # Weights arrive [co, ci, du, dv]; matmuls need [ci, co] per tap.
        # A strided gather DMA would be 4-byte-descriptor-bound, so load
        # contiguously and transpose on the PE instead.
        from concourse.masks import make_identity

        ident = stage.tile([128, 128], F32)
        make_identity(nc, ident[:])

        w1raw = stage.tile([CH, 9], F32)
        nc.sync.dma_start(out=w1raw[:], in_=_ap(self.w1, 0, [(9, CH), (1, 9)]))
        w2raw = stage.tile([CH, 9 * CH], F32)
        nc.sync.dma_start(out=w2raw[:], in_=_ap(self.w2, 0, [(9 * CH, CH), (1, 9 * CH)]))
        w3raw_a = stage.tile([128, 9 * CH], F32)
        nc.sync.dma_start(
            out=w3raw_a[:], in_=_ap(self.w3, 0, [(9 * CH, 128), (1, 9 * CH)])
        )
        w3raw_b = stage.tile([CHI, 9 * CH], F32)
        nc.sync.dma_start(
            out=w3raw_b[:],
            in_=_ap(self.w3, 128 * 9 * CH, [(9 * CH, CHI), (1, 9 * CH)]),
        )

        def tapv(raw, t, n):  # [n_co, ci] view of tap t
            return raw[0:n, :].rearrange("p (ci t) -> p t ci", t=9)[:, t, :]

        # w1: lhsT [9 taps, 64 co]; copy at partitions 64-72 for the
        # row-tiled chunk-b matmul (lhs/rhs start partitions must match)
        pT = pwtr.tile([128, 128], F32, tag="ps3lo")
        nc.tensor.transpose(pT[0:9, 0:CH], w1raw[:], ident[0:CH, 0:CH])
        self.w1sb = consts.tile([128, CH], F32R)
        nc.vector.tensor_copy(self.w1sb[0:9, :], pT[0:9, 0:CH])
        nc.sync.dma_start(out=self.w1sb[64:73, :], in_=self.w1sb[0:9, :])

        # Transpose each tap to PSUM base 0 (HW requires base 0); upper
        # (shifted-partner tap) halves staged then partition-shifted to
        # partitions 64-127 by one SBUF->SBUF DMA per weight tile.
        self.w2p = consts.tile([2 * CH, 3 * CH], F32R)
        self.w2s = consts.tile([CH, 3 * CH], F32R)
        # conv3: A-pairs (du,0)+(du,1) du=0..2; B-pair (0,2)+(1,2);
        # single (2,2); each split into co groups [0:128] / [128:169]
        self.w3pA_lo = consts.tile([2 * CH, 3 * CLO], F32R)
        self.w3pA_hi = consts.tile([2 * CH, 3 * CHI], F32R)
        self.w3pB_lo = consts.tile([2 * CH, CLO], F32R)
        self.w3pB_hi = consts.tile([2 * CH, CHI], F32R)
        self.w3s_lo = consts.tile([CH, CLO], F32R)
        self.w3s_hi = consts.tile([CH, CHI], F32R)
        w2pu = stage.tile([CH, 3 * CH], F32R)
        w3puA_lo = stage.tile([CH, 3 * CLO], F32R)
        w3puA_hi = stage.tile([CH, 3 * CHI], F32R)
        w3puB_lo = stage.tile([CH, CLO], F32R)
        w3puB_hi = stage.tile([CH, CHI], F32R)

        def tr(dst, raw, t, n):
            pT = pwtr.tile([CH, 128], F32, tag="ps3lo")
            nc.tensor.transpose(pT[:, 0:n], tapv(raw, t, n), ident[0:n, 0:n])
            nc.vector.tensor_copy(dst, pT[:, 0:n])

        for p in range(3):
            cw = slice(p * CH, (p + 1) * CH)
            cl = slice(p * CLO, (p + 1) * CLO)
            ch = slice(p * CHI, (p + 1) * CHI)
            tr(self.w2p[0:CH, cw], w2raw, p * 3, CH)
            tr(w2pu[:, cw], w2raw, p * 3 + 1, CH)
            tr(self.w2s[:, cw], w2raw, p * 3 + 2, CH)
            tr(self.w3pA_lo[0:CH, cl], w3raw_a, p * 3, 128)
            tr(self.w3pA_hi[0:CH, ch], w3raw_b, p * 3, CHI)
            tr(w3puA_lo[:, cl], w3raw_a, p * 3 + 1, 128)
            tr(w3puA_hi[:, ch], w3raw_b, p * 3 + 1, CHI)
        tr(self.w3pB_lo[0:CH, :], w3raw_a, 2, 128)
        tr(self.w3pB_hi[0:CH, :], w3raw_b, 2, CHI)
        tr(w3puB_lo[:], w3raw_a, 5, 128)
        tr(w3puB_hi[:], w3raw_b, 5, CHI)
        tr(self.w3s_lo[:], w3raw_a, 8, 128)
        tr(self.w3s_hi[:], w3raw_b, 8, CHI)
        nc.sync.dma_start(out=self.w2p[CH:, :], in_=w2pu[:])
        nc.sync.dma_start(out=self.w3pA_lo[CH:, :], in_=w3puA_lo[:])
        nc.sync.dma_start(out=self.w3pA_hi[CH:, :], in_=w3puA_hi[:])
        nc.sync.dma_start(out=self.w3pB_lo[CH:, :], in_=w3puB_lo[:])
        nc.sync.dma_start(out=self.w3pB_hi[CH:, :], in_=w3puB_hi[:])

        # biases replicated into partitions 64-127 for the chunk-b relus
        # (engine lanes are partition-hardwired)
        self.b1sb = consts.tile([2 * CH, 1], F32)
        nc.scalar.dma_start(out=self.b1sb[0:CH], in_=_ap(self.b1, 0, [(1, CH), (0, 1)]))
        nc.scalar.dma_start(out=self.b1sb[CH:], in_=_ap(self.b1, 0, [(1, CH), (0, 1)]))
        self.b2sb = consts.tile([2 * CH, 1], F32)
        nc.scalar.dma_start(out=self.b2sb[0:CH], in_=_ap(self.b2, 0, [(1, CH), (0, 1)]))
        nc.scalar.dma_start(out=self.b2sb[CH:], in_=_ap(self.b2, 0, [(1, CH), (0, 1)]))
        self.b3lo = consts.tile([CLO, 1], F32)
        nc.scalar.dma_start(out=self.b3lo[:], in_=_ap(self.b3, 0, [(1, CLO), (0, 1)]))
        self.b3hi = consts.tile([CHI, 1], F32)
        nc.scalar.dma_start(out=self.b3hi[:], in_=_ap(self.b3, CLO, [(1, CHI), (0, 1)]))

        # staircase-ones: stair[:, 128] = 1, else 0; column j of the view
        # stair[:, 128-j : 192-j] is all-ones -> matmul writes the partition
        # sum into PSUM row j (zeros elsewhere, harmless under accumulation)
        stair_st = stage.tile([128, 256], F32)
        nc.vector.memset(stair_st[:], 0.0)
        nc.vector.memset(stair_st[:, 128:129], 1.0)
        self.stair = consts.tile([128, 256], F32R)
        nc.vector.tensor_copy(self.stair[:], stair_st[:])

        stage.release()
        stage2 = tc.alloc_tile_pool(name="stage2", bufs=1)

        # x -> fp32r copy in DRAM (conv1 im2col source) and bf16 copy (xs
        # gather source: xs feeds only the DVE, so half-width is fine)
        xst = stage2.tile([128, H * W // 128], F32)
        nc.sync.dma_start(
            out=xst[:], in_=_ap(self.x, 0, [(H * W // 128, 128), (1, H * W // 128)])
        )
        xsr = stage2.tile([128, H * W // 128], F32R)
        nc.vector.tensor_copy(xsr[:], xst[:])
        nc.sync.dma_start(
            out=_ap(self.xr, 0, [(H * W // 128, 128), (1, H * W // 128)]), in_=xsr[:]
        )
        xsb = stage2.tile([128, H * W // 128], BF16)
        nc.vector.tensor_copy(xsb[:], xst[:])
        nc.sync.dma_start(
            out=_ap(self.xb, 0, [(H * W // 128, 128), (1, H * W // 128)]), in_=xsb[:]
        )
        stage2.release()

    # ---------------- per-strip stages ----------------

    def emit_conv1(self, i0, S):
        # Chunk-paired via PE array tiling: chunk a (first half-strip) runs
        # in tile (0,0) [SBUF 0-31 -> PSUM 0-63], chunk b (second half) in
        # tile (64,64) [SBUF 64-95 -> PSUM 64-127], concurrently. relu-a
        # writes h lower; relu-b (lanes 64-127) writes h upper pre-shifted;
        # two coarse dup DMAs fill in the opposite halves.
        #
        # im2col DMAs for the whole strip are issued up-front (the 5-deep
        # imc pool holds a full strip) so the PE never waits on DMA latency
        # when the deferred matmul bursts run a strip later.
        nc = self.nc
        c0 = i0 + 6
        L1 = (S + 6) * W
        h1t = self.p_h1.tile([2 * CH, (S_STRIP + 6) * W + 772], F32R, tag="h1")
        nc.gpsimd.memset(h1t[0:CH, L1 : L1 + 772].bitcast(F32), 0.0)
        nc.gpsimd.memset(h1t[CH:, L1 - 1 : L1 + 771].bitcast(F32), 0.0)
        Lh = (L1 // (2 * NC_)) * NC_
        groups = list(range(0, L1, 2 * NC_))
        imcs = {}

        def emit_dmas():
            for hs in groups:
                he = min(hs + 2 * NC_, L1)
                imc = self.p_imc.tile([9, 2 * NC_], F32R, tag="imc")
                nc.sync.dma_start(
                    out=imc[:, 0 : he - hs],
                    in_=_ap(self.xr, (c0 - 5) * W - 1 + hs,
                            [(W, 3), (1, 3), (1, he - hs)]),
                )
                imcs[hs] = imc

        def emit_groups(grps):
            for hs in grps:
                he = min(hs + 2 * NC_, L1)
                imc = imcs[hs]
                for cs in range(hs, he, NC_):
                    ce = min(cs + NC_, L1)
                    pt = self.ps12.tile([CH, NC_], F32, tag="ps12")
                    _mm(nc, pt[:, 0 : ce - cs], self.w1sb[0:9, :],
                        imc[:, cs - hs : ce - hs], True, True)
                    nc.scalar.activation(
                        h1t[0:CH, cs:ce], pt[:, 0 : ce - cs], AF.Relu,
                        bias=self.b1sb[0:CH],
                    )
                    dup = nc.gpsimd if DUP_GP else nc.sync
                    if ce == Lh:
                        dup.dma_start(out=h1t[CH:, 0 : Lh - 1], in_=h1t[0:CH, 1:Lh])
                    elif ce == L1:
                        dup.dma_start(
                            out=h1t[CH:, Lh - 1 : L1 - 1], in_=h1t[0:CH, Lh:L1]
                        )

        # split into three bursts so conv1's scalar-relu chain (684ns vs
        # ~290ns mm) doesn't back up the in-order PE queue in one long run
        return (h1t, emit_dmas, lambda: emit_groups(groups[:2]),
                lambda: emit_groups(groups[2:4]),
                lambda: emit_groups(groups[4:]))

    def emit_xs(self, i0, S):
        # xs[(u,v), i*W + j] = x[i0+u+i, j+v]: one contiguous read per
        # tap-row u (13 partitions x (S-1)*W+244 elements) into the spaced
        # layout; cols 244..256 of each row hold neighbor-row junk that the
        # stt views never touch. Tap-row u=9 straddles the 128-tap split:
        # taps 117..127 land in xs_lo[117:128], taps 128..129 in xs_hi[0:2].
        nc = self.nc
        LS = (S - 1) * W + HO
        xs_lo = self.p_xsl.tile([CLO, S_STRIP * W], BF16, tag="xsl")
        eng_lo = nc.gpsimd if XS_GP else nc.scalar
        eng_hi = nc.gpsimd if XS_GP else nc.sync
        for u in range(9):
            eng_lo.dma_start(
                out=xs_lo[u * K : (u + 1) * K, 0:LS],
                in_=_ap(self.xb, (i0 + u) * W, [(1, K), (1, LS)]),
            )
        eng_lo.dma_start(
            out=xs_lo[117:128, 0:LS],
            in_=_ap(self.xb, (i0 + 9) * W, [(1, 11), (1, LS)]),
        )
        xs_hi = self.p_xsh.tile([CHI, S_STRIP * W], BF16, tag="xsh")
        eng_hi.dma_start(
            out=xs_hi[0:2, 0:LS],
            in_=_ap(self.xb, (i0 + 9) * W + 11, [(1, 2), (1, LS)]),
        )
        for u in range(10, 13):
            eng_hi.dma_start(
                out=xs_hi[2 + (u - 10) * K : 2 + (u - 9) * K, 0:LS],
                in_=_ap(self.xb, (i0 + u) * W, [(1, K), (1, LS)]),
            )
        return xs_lo, xs_hi

    def emit_conv2(self, i0, S, h1t):
        # Alternate chunk parity between [pairs K=128; singles K=64] and
        # [singles; pairs] so same-row-size groups meet across chunk
        # boundaries: one PE row-size-transition drain per chunk, not two.
        # Also builds the conv3 B tile h2b = [h2; h2<<W] via two dup DMAs
        # per half-strip (four total).
        nc = self.nc
        L2 = (S + 3) * W
        LB = S * W + 320
        h2t = self.p_h2.tile([2 * CH, (S_STRIP + 3) * W + 772], F32R, tag="h2")
        h2b = self.p_h2b.tile([2 * CH, S_STRIP * W + 320], F32R, tag="h2b")
        nc.gpsimd.memset(h2t[0:CH, L2 : L2 + 772].bitcast(F32), 0.0)
        nc.gpsimd.memset(h2t[CH:, L2 - 1 : L2 + 771].bitcast(F32), 0.0)
        Lh = (L2 // (2 * NC_)) * NC_
        for ci, cs in enumerate(range(0, L2, NC_)):
            ce = min(cs + NC_, L2)
            pt = self.ps12.tile([CH, NC_], F32, tag="ps12")

            def pairs(first):
                for p in range(3):
                    off = p * W + 255
                    _mm(nc, pt[:, 0 : ce - cs],
                        self.w2p[:, p * CH : (p + 1) * CH],
                        h1t[:, cs + off : ce + off], first and p == 0,
                        not first and p == 2)

            def singles(first):
                for p in range(3):
                    off = p * W + 2 + 255
                    _mm(nc, pt[:, 0 : ce - cs],
                        self.w2s[:, p * CH : (p + 1) * CH],
                        h1t[0:CH, cs + off : ce + off], first and p == 0,
                        not first and p == 2)

            if ci % 2 == 0:
                pairs(True)
                singles(False)
            else:
                singles(True)
                pairs(False)
            nc.scalar.activation(
                h2t[0:CH, cs:ce], pt[:, 0 : ce - cs], AF.Relu, bias=self.b2sb[0:CH]
            )
            dup = nc.scalar if DUP_GP else nc.sync
            if ce == Lh:
                dup.dma_start(out=h2t[CH:, 0 : Lh - 1], in_=h2t[0:CH, 1:Lh])
                dup.dma_start(out=h2b[0:CH, 0:Lh], in_=h2t[0:CH, 0:Lh])
                dup.dma_start(out=h2b[CH:, 0 : Lh - W], in_=h2t[0:CH, W:Lh])
            elif ce == L2:
                dup.dma_start(out=h2t[CH:, Lh - 1 : L2 - 1], in_=h2t[0:CH, Lh:L2])
                dup.dma_start(out=h2b[0:CH, Lh:LB], in_=h2t[0:CH, Lh:LB])
                dup.dma_start(
                    out=h2b[CH:, Lh - W : LB], in_=h2t[0:CH, Lh : LB + W]
                )
        return h2t, h2b

    def emit_conv3_chunk(self, i0, cs, h2t, h2b, xs_lo, xs_hi, flush):
        """conv3 + stt for one 2-row chunk; staircase matmuls are deferred.

        Chunk parity alternates [singles K=64 | flush+pairs K=128] and
        [pairs | flush | singles] so consecutive chunks share row sizes at
        their boundary: one PE drain per chunk. The deferred stair flush
        (K=128) is injected inside the K=128 run."""
        nc = self.nc
        even = self.gchunk % 2 == 0
        self.gchunk += 1
        plo = self.ps3.tile([CLO, NC_], F32, tag="ps3lo")
        phi = self.ps3.tile([CHI, NC_], F32, tag="ps3hi")

        def singles(first, stop):
            off = 2 * W + 2 + 255
            rhs = h2t[0:CH, cs + off : cs + NC_ + off]
            _mm(nc, plo[:], self.w3s_lo[:], rhs, first, stop)
            _mm(nc, phi[:], self.w3s_hi[:], rhs, first, stop)

        def pairs(first, stop):
            for p in range(3):
                off = p * W + 255
                rhs = h2t[:, cs + off : cs + NC_ + off]
                _mm(nc, plo[:], self.w3pA_lo[:, p * CLO : (p + 1) * CLO],
                    rhs, first and p == 0, False)
                _mm(nc, phi[:], self.w3pA_hi[:, p * CHI : (p + 1) * CHI],
                    rhs, first and p == 0, False)
            offb = 2 + 255
            rhsb = h2b[:, cs + offb : cs + NC_ + offb]
            _mm(nc, plo[:], self.w3pB_lo[:], rhsb, False, stop)
            _mm(nc, phi[:], self.w3pB_hi[:], rhsb, False, stop)

        flush_hi, flush_lo = flush
        if even:
            singles(True, False)
            flush_hi()
            flush_lo()
            pairs(False, True)
        else:
            pairs(True, False)
            flush_lo()
            singles(False, True)
            flush_hi()

        r2 = cs // W
        jj = (i0 + r2) // 2
        # t2 = (conv3_psum + b3) * xs, straight from PSUM on the DVE; then
        # gpsimd folds the 41 hi-tap products into t2[0:41] so the stair
        # reduction is a single K=128 matmul.
        t2 = self.p_t2.tile([CLO, NC_], F32R, tag="t2")
        t_hi = self.p_th.tile([CHI, NC_], F32R, tag="t_hi")
        wv_lo = plo[:].rearrange("p (r c) -> p r c", c=W)[:, :, 6 : 6 + HO]
        wv_hi = phi[:].rearrange("p (r c) -> p r c", c=W)[:, :, 6 : 6 + HO]
        xv_lo = xs_lo[:, cs : cs + NC_].rearrange("p (r c) -> p r c", c=W)[:, :, 0:HO]
        xv_hi = xs_hi[:, cs : cs + NC_].rearrange("p (r c) -> p r c", c=W)[:, :, 0:HO]
        tv_lo = t2[:].rearrange("p (r c) -> p r c", c=W)[:, :, 0:HO]
        tv_hi = t_hi[:].rearrange("p (r c) -> p r c", c=W)[:, :, 0:HO]
        nc.vector.scalar_tensor_tensor(
            out=tv_lo, in0=wv_lo, scalar=self.b3lo[:], in1=xv_lo,
            op0=ALU.add, op1=ALU.mult,
        )
        nc.vector.scalar_tensor_tensor(
            out=tv_hi, in0=wv_hi, scalar=self.b3hi[:], in1=xv_hi,
            op0=ALU.add, op1=ALU.mult,
        )
        self.pend_lo.append((t2, jj))
        self.pend_hi.append((t_hi, jj))

    def _stair_mm(self, t_t, np_, jj):
        # psum_y is split into two [64, 488] banks so the staircase lhsT is
        # only M=64 columns (stationary load halves; only one col is ones).
        # Each 2-row chunk jj lands as TWO stair matmuls: K=128 over t2 and
        # K=41 over t_hi (rounds to the singles' 64-row tile config, so each
        # flush sits transition-free inside its same-row-size region).
        nc = self.nc
        rhs = t_t[0:np_].rearrange("p (r c) -> p r c", c=W)[:, :, 0:HO]
        if jj < 64:
            _mm(nc, self.psum_ya[:], self.stair[0:np_, 128 - jj : 192 - jj],
                rhs, self.cnt_a == 0, self.cnt_a == 2 * 64 - 1)
            self.cnt_a += 1
        else:
            _mm(nc, self.psum_yb[:], self.stair[0:np_, 192 - jj : 256 - jj],
                rhs, self.cnt_b == 0, self.cnt_b == 2 * (self.NYC - 64) - 1)
            self.cnt_b += 1

    def flush_stair_lo(self, keep=0):
        while len(self.pend_lo) > keep:
            t2, jj = self.pend_lo.pop(0)
            self._stair_mm(t2, CLO, jj)

    def flush_stair_hi(self, keep=0):
        while len(self.pend_hi) > keep:
            t_hi, jj = self.pend_hi.pop(0)
            self._stair_mm(t_hi, CHI, jj)

    def _build_strips(self):
        nc = self.nc
        self.NYC = (HO * HO) // 488  # 122
        self.psum_ya = self.psy.tile([64, 488], F32, tag="ya")
        self.psum_yb = self.psy.tile([64, 488], F32, tag="yb")
        self.pend_lo = []
        self.pend_hi = []
        self.cnt_a = 0
        self.cnt_b = 0
        self.gchunk = 0

        strips = []
        i0 = 0
        while i0 < HO:
            strips.append((i0, min(S_STRIP, HO - i0)))
            i0 += S_STRIP

        h1t, c1d, c1a, c1b, c1c = self.emit_conv1(*strips[0])
        c1d()
        c1a()
        c1b()
        c1c()
        xs = self.emit_xs(*strips[0])
        keep = 4 if PIPE else 0
        flush = (lambda: self.flush_stair_hi(keep=keep),
                 lambda: self.flush_stair_lo(keep=keep))
        for si, (i0, S) in enumerate(strips):
            h2t, h2b = self.emit_conv2(i0, S, h1t)
            xs_lo, xs_hi = xs
            # prefetch next strip's xs while this strip's conv3 runs
            if si + 1 < len(strips):
                xs = self.emit_xs(*strips[si + 1])
                h1n, c1d, c1a, c1b, c1c = self.emit_conv1(*strips[si + 1])
                c1d()
            else:
                c1a = c1b = c1c = None
            for ci, cs in enumerate(range(0, S * W, NC_)):
                self.emit_conv3_chunk(
                    i0, cs, h2t, h2b, xs_lo, xs_hi, flush=flush,
                )
                # overlap next strip's conv1 with this strip's conv3 tail,
                # split into three bursts
                if PIPE and ci == 1 and c1a is not None:
                    c1a()
                    c1a = None
                if PIPE and ci == 3 and c1b is not None:
                    c1b()
                    c1b = None
                if PIPE and ci == 5 and c1c is not None:
                    c1c()
                    c1c = None
            for fn in (c1a, c1b, c1c):
                if fn is not None:
                    fn()
            if si + 1 < len(strips):
                h1t = h1n
            self.flush_stair_hi(keep=0)
            self.flush_stair_lo(keep=0)

        ysba = self.p_y.tile([64, 488], F32, tag="ya")
        nc.vector.tensor_copy(ysba[:], self.psum_ya[:])
        nc.sync.dma_start(out=_ap(self.y, 0, [(488, 64), (1, 488)]), in_=ysba[:])
        nb = self.NYC - 64  # 58
        ysbb = self.p_y.tile([64, 488], F32, tag="ya")
        nc.vector.tensor_copy(ysbb[0:nb, :], self.psum_yb[0:nb, :])
        nc.sync.dma_start(
            out=_ap(self.y, 64 * 488, [(488, nb), (1, 488)]), in_=ysbb[0:nb, :]
        )


_NC_CACHE = {}


def _get_nc():
    if "nc" not in _NC_CACHE:
        _NC_CACHE["nc"] = build_nc()
    return _NC_CACHE["nc"]


def _in_maps(inputs):
    x = np.ascontiguousarray(np.asarray(inputs["x"], dtype=np.float32))
    names = ["W1", "b1", "W2", "b2", "W3", "b3"]
    ws = {n: np.ascontiguousarray(np.asarray(inputs[n], np.float32)) for n in names}
    maps = []
    for i in range(8):
        m = {"x": x[i : i + 1]}
        m.update(ws)
        maps.append(m)
    return maps


def kernel(**inputs):
    nc = _get_nc()
    res = run_bass_kernel_spmd(nc, _in_maps(inputs), list(range(8)))
    return np.concatenate([res.results[i]["y"] for i in range(8)], axis=0)


def profile(**inputs):
    nc = _get_nc()
    res = run_bass_kernel_spmd(nc, _in_maps(inputs), list(range(8)), trace=True)
    return res.exec_time_ns


if __name__ == "__main__":
    rng = np.random.RandomState(0)
    ins = {
        "x": rng.randn(8, 1, H, W).astype(np.float32),
        "W1": rng.randn(CH, 1, 3, 3).astype(np.float32) * 0.1,
        "b1": np.zeros(CH, np.float32),
        "W2": rng.randn(CH, CH, 3, 3).astype(np.float32) * 0.05,
        "b2": np.zeros(CH, np.float32),
        "W3": rng.randn(C3, CH, 3, 3).astype(np.float32) * 0.05,
        "b3": np.zeros(C3, np.float32),
    }
    print(kernel(**ins).shape)


# revision 18
# speedup vs baseline: 1.3146x; 1.3146x over previous
"""NlmCNN (weight-predicting CNN + per-pixel 13x13 weighted sum) on 8 trn2 cores.

Sharding: data-parallel over batch (8 images -> 8 cores), weights replicated.

Per-core layout trick: output y is the conv stack's result center-cropped by
6 pixels, and the receptive field of the three 3x3 convs only reaches 3 px
out, so SAME-padding never materializes: every layer is computed VALID-style
on an unpadded 256-stride flat layout. Column-wrap junk from flat shifted
reads stays confined to the outer <=3 columns of each layer, which are
discarded by the crop.

All matmul operands are bf16 (fp32 PSUM accumulation; end-to-end absmax-rel
~4e-3 vs the 2e-2 gate). bf16 is chosen over float32r because fp32-class
LDWEIGHTS runs ~2.2ns/stationary-column with FWL disabled: M=128 weight
loads (285ns) exceed the N=512 stream time (213ns) and the PE becomes
weight-load-bound (measured 426ns/matmul cadence). bf16 enables FWL and
streams the same 1 column/cycle.

Pipeline per strip of S output rows (strips software-pipelined: conv1 of
strip i+1 is emitted during strip i's conv3 phase):
  conv1: per-2-chunk im2col [9, 1024] via one 3-dim DMA -> K=9 matmul; all
         of a strip's im2col DMAs are issued a strip ahead (the imc pool
         holds a full strip) so the PE never waits on DMA latency.
  conv2/conv3: 3x3 taps packed into K=128 pair-matmuls: taps (du,0)+(du,1)
         pair on hA=[h; h<<1] (upper 64 partitions hold h shifted +1);
         taps (0,2)+(1,2) pair on hB=[h; h<<W]; only tap (2,2) is a K=64
         single. 5 matmuls per 512-px chunk for conv2, 10 for conv3
         (out channels split [0:128] M=128 / [128:169] M=41). The shifted
         tiles are built by SBUF->SBUF dup DMAs per half-strip. Chunk
         parity alternates [single | pairs] / [pairs | single] so the PE
         sees one K-row-size transition per chunk.
  einsum: patch matrix xs[t, s] = x[pos + shift(t)] split [128 | 41] taps,
         gathered bf16 by one contiguous DMA per tap-row u (row u=9 is
         split 11/2 across the two tiles); DVE scalar_tensor_tensor
         computes t2 = (conv3_lo + b3_lo) * xs_lo and
         t_hi = (conv3_hi + b3_hi) * xs_hi straight from PSUM; the
         partition reduction is "staircase ones" matmuls (K=128 over t2
         next to the pairs, K=41 over t_hi next to the K=64 singles --
         both transition-free), accumulating 2-row chunk jj into row jj
         of a persistent PSUM tile; one copy + one DMA store the image.
"""

import numpy as np

import concourse.bacc as bacc
import concourse.bass as bass
import concourse.mybir as mybir
import concourse.tile as tile
from concourse.bass_utils import run_bass_kernel_spmd

F32 = mybir.dt.float32
BF16 = mybir.dt.bfloat16
AF = mybir.ActivationFunctionType
ALU = mybir.AluOpType

H = 256
W = 256
K = 13
HO = H - K + 1  # 244
CH = 64
C3 = K * K  # 169
CLO = 128   # conv3 out-channel group sizes
CHI = C3 - 128  # 41
S_STRIP = 16
NC_ = 512  # chunk positions (2 image rows)
import os
PIPE = os.environ.get("K_PIPE", "1") == "1"      # cross-strip sw pipelining
XS_GP = os.environ.get("K_XS_GP", "1") == "1"    # xs DMAs on gpsimd queue
DUP_GP = os.environ.get("K_DUP_GP", "1") == "1"  # dup DMAs on gpsimd queue


def _ap(t, off, dims):
    return bass.AP(t, off, [list(d) for d in dims])


def _mm(nc, out, lhsT, rhs, start, stop):
    nc.tensor.matmul(out, lhsT, rhs, start=start, stop=stop)


def build_nc():
    nc = bacc.Bacc("TRN2", target_bir_lowering=False, debug=False)

    x = nc.dram_tensor("x", [1, 1, H, W], F32, kind="ExternalInput")
    w1 = nc.dram_tensor("W1", [CH, 1, 3, 3], F32, kind="ExternalInput")
    b1 = nc.dram_tensor("b1", [CH], F32, kind="ExternalInput")
    w2 = nc.dram_tensor("W2", [CH, CH, 3, 3], F32, kind="ExternalInput")
    b2 = nc.dram_tensor("b2", [CH], F32, kind="ExternalInput")
    w3 = nc.dram_tensor("W3", [C3, CH, 3, 3], F32, kind="ExternalInput")
    b3 = nc.dram_tensor("b3", [C3], F32, kind="ExternalInput")
    y = nc.dram_tensor("y", [1, 1, HO, HO], F32, kind="ExternalOutput")
    xb = nc.dram_tensor("x_b", [H * W], BF16)

    with tile.TileContext(nc) as tc:
        Body(nc, tc, x, w1, b1, w2, b2, w3, b3, y, xb).build()

    nc.compile()
    return nc


class Body:
    def __init__(self, nc, tc, x, w1, b1, w2, b2, w3, b3, y, xb):
        self.nc, self.tc = nc, tc
        self.x, self.w1, self.b1, self.w2, self.b2 = x, w1, b1, w2, b2
        self.w3, self.b3, self.y, self.xb = w3, b3, y, xb

    def build(self):
        nc, tc = self.nc, self.tc
        with (
            tc.tile_pool(name="consts", bufs=1) as consts,
            tc.tile_pool(name="t2p", bufs=5) as p_t2,
            tc.tile_pool(name="thp", bufs=5) as p_th,
            tc.tile_pool(name="imc", bufs=6) as p_imc,
            tc.tile_pool(name="h1p", bufs=2) as p_h1,
            tc.tile_pool(name="h1bp", bufs=2) as p_h1b,
            tc.tile_pool(name="h2p", bufs=2) as p_h2,
            tc.tile_pool(name="h2bp", bufs=2) as p_h2b,
            tc.tile_pool(name="xsl", bufs=2) as p_xsl,
            tc.tile_pool(name="xsh", bufs=2) as p_xsh,
            tc.tile_pool(name="yout", bufs=1) as p_y,
            tc.tile_pool(name="ps12", bufs=2, space="PSUM") as ps12,
            tc.tile_pool(name="ps3", bufs=2, space="PSUM") as ps3,
            tc.tile_pool(name="psy", bufs=1, space="PSUM") as psy,
        ):
            self.consts = consts
            self.p_t2, self.p_th, self.p_imc = p_t2, p_th, p_imc
            self.p_h1, self.p_h1b = p_h1, p_h1b
            self.p_h2, self.p_h2b = p_h2, p_h2b
            self.p_xsl, self.p_xsh = p_xsl, p_xsh
            self.p_y, self.ps12, self.ps3, self.psy = p_y, ps12, ps3, psy
            self._build_consts()
            self._build_strips()

    def _build_consts(self):
        nc, tc, consts = self.nc, self.tc, self.consts
        stage = tc.alloc_tile_pool(name="stage", bufs=1)
        # weight-prep transposes borrow ps3's "ps3lo" slot (same max tile
        # size, consts-time only) so no dedicated PSUM bank is needed
        pwtr = self.ps3

        # Weights arrive [co, ci, du, dv]; matmuls need [ci, co] per tap.
        # A strided gather DMA would be 4-byte-descriptor-bound, so load
        # contiguously and transpose on the PE instead.
        from concourse.masks import make_identity

        ident = stage.tile([128, 128], F32)
        make_identity(nc, ident[:])

        w1raw = stage.tile([CH, 9], F32)
        nc.sync.dma_start(out=w1raw[:], in_=_ap(self.w1, 0, [(9, CH), (1, 9)]))
        w2raw = stage.tile([CH, 9 * CH], F32)
        nc.sync.dma_start(out=w2raw[:], in_=_ap(self.w2, 0, [(9 * CH, CH), (1, 9 * CH)]))
        w3raw_a = stage.tile([128, 9 * CH], F32)
        nc.sync.dma_start(
            out=w3raw_a[:], in_=_ap(self.w3, 0, [(9 * CH, 128), (1, 9 * CH)])
        )
        w3raw_b = stage.tile([CHI, 9 * CH], F32)
        nc.sync.dma_start(
            out=w3raw_b[:],
            in_=_ap(self.w3, 128 * 9 * CH, [(9 * CH, CHI), (1, 9 * CH)]),
        )

        def tapv(raw, t, n):  # [n_co, ci] view of tap t
            return raw[0:n, :].rearrange("p (ci t) -> p t ci", t=9)[:, t, :]

        # w1: lhsT [9 taps, 64 co]; copy at partitions 64-72 for the
        # row-tiled chunk-b matmul (lhs/rhs start partitions must match)
        pT = pwtr.tile([128, 128], F32, tag="ps3lo")
        nc.tensor.transpose(pT[0:9, 0:CH], w1raw[:], ident[0:CH, 0:CH])
        self.w1sb = consts.tile([128, CH], BF16)
        nc.vector.tensor_copy(self.w1sb[0:9, :], pT[0:9, 0:CH])
        nc.sync.dma_start(out=self.w1sb[64:73, :], in_=self.w1sb[0:9, :])

        # Transpose each tap to PSUM base 0 (HW requires base 0); upper
        # (shifted-partner tap) halves staged then partition-shifted to
        # partitions 64-127 by one SBUF->SBUF DMA per weight tile.
        # A-pairs carry taps (du,0)+(du,1) du=0..2; B-pair (0,2)+(1,2);
        # single is tap (2,2); conv3 splits co into [0:128] / [128:169].
        self.w2p = consts.tile([2 * CH, 3 * CH], BF16)
        self.w2pB = consts.tile([2 * CH, CH], BF16)
        self.w2s = consts.tile([CH, CH], BF16)
        self.w3pA_lo = consts.tile([2 * CH, 3 * CLO], BF16)
        self.w3pA_hi = consts.tile([2 * CH, 3 * CHI], BF16)
        self.w3pB_lo = consts.tile([2 * CH, CLO], BF16)
        self.w3pB_hi = consts.tile([2 * CH, CHI], BF16)
        self.w3s_lo = consts.tile([CH, CLO], BF16)
        self.w3s_hi = consts.tile([CH, CHI], BF16)
        w2pu = stage.tile([CH, 3 * CH], BF16)
        w2puB = stage.tile([CH, CH], BF16)
        w3puA_lo = stage.tile([CH, 3 * CLO], BF16)
        w3puA_hi = stage.tile([CH, 3 * CHI], BF16)
        w3puB_lo = stage.tile([CH, CLO], BF16)
        w3puB_hi = stage.tile([CH, CHI], BF16)

        def tr(dst, raw, t, n):
            pT = pwtr.tile([CH, 128], F32, tag="ps3lo")
            nc.tensor.transpose(pT[:, 0:n], tapv(raw, t, n), ident[0:n, 0:n])
            nc.vector.tensor_copy(dst, pT[:, 0:n])

        for p in range(3):
            cw = slice(p * CH, (p + 1) * CH)
            cl = slice(p * CLO, (p + 1) * CLO)
            ch = slice(p * CHI, (p + 1) * CHI)
            tr(self.w2p[0:CH, cw], w2raw, p * 3, CH)
            tr(w2pu[:, cw], w2raw, p * 3 + 1, CH)
            tr(self.w3pA_lo[0:CH, cl], w3raw_a, p * 3, 128)
            tr(self.w3pA_hi[0:CH, ch], w3raw_b, p * 3, CHI)
            tr(w3puA_lo[:, cl], w3raw_a, p * 3 + 1, 128)
            tr(w3puA_hi[:, ch], w3raw_b, p * 3 + 1, CHI)
        tr(self.w2pB[0:CH, :], w2raw, 2, CH)
        tr(w2puB[:], w2raw, 5, CH)
        tr(self.w2s[:], w2raw, 8, CH)
        tr(self.w3pB_lo[0:CH, :], w3raw_a, 2, 128)
        tr(self.w3pB_hi[0:CH, :], w3raw_b, 2, CHI)
        tr(w3puB_lo[:], w3raw_a, 5, 128)
        tr(w3puB_hi[:], w3raw_b, 5, CHI)
        tr(self.w3s_lo[:], w3raw_a, 8, 128)
        tr(self.w3s_hi[:], w3raw_b, 8, CHI)
        nc.sync.dma_start(out=self.w2p[CH:, :], in_=w2pu[:])
        nc.sync.dma_start(out=self.w2pB[CH:, :], in_=w2puB[:])
        nc.sync.dma_start(out=self.w3pA_lo[CH:, :], in_=w3puA_lo[:])
        nc.sync.dma_start(out=self.w3pA_hi[CH:, :], in_=w3puA_hi[:])
        nc.sync.dma_start(out=self.w3pB_lo[CH:, :], in_=w3puB_lo[:])
        nc.sync.dma_start(out=self.w3pB_hi[CH:, :], in_=w3puB_hi[:])

        # biases replicated into partitions 64-127 for the chunk-b relus
        # (engine lanes are partition-hardwired)
        self.b1sb = consts.tile([2 * CH, 1], F32)
        nc.scalar.dma_start(out=self.b1sb[0:CH], in_=_ap(self.b1, 0, [(1, CH), (0, 1)]))
        nc.scalar.dma_start(out=self.b1sb[CH:], in_=_ap(self.b1, 0, [(1, CH), (0, 1)]))
        self.b2sb = consts.tile([2 * CH, 1], F32)
        nc.scalar.dma_start(out=self.b2sb[0:CH], in_=_ap(self.b2, 0, [(1, CH), (0, 1)]))
        nc.scalar.dma_start(out=self.b2sb[CH:], in_=_ap(self.b2, 0, [(1, CH), (0, 1)]))
        self.b3lo = consts.tile([CLO, 1], F32)
        nc.scalar.dma_start(out=self.b3lo[:], in_=_ap(self.b3, 0, [(1, CLO), (0, 1)]))
        self.b3hi = consts.tile([CHI, 1], F32)
        nc.scalar.dma_start(out=self.b3hi[:], in_=_ap(self.b3, CLO, [(1, CHI), (0, 1)]))

        # staircase-ones: stair[:, 128] = 1, else 0; column j of the view
        # stair[:, 128-j : 192-j] is all-ones -> matmul writes the partition
        # sum into PSUM row j (zeros elsewhere, harmless under accumulation)
        stair_st = stage.tile([128, 256], F32)
        nc.vector.memset(stair_st[:], 0.0)
        nc.vector.memset(stair_st[:, 128:129], 1.0)
        self.stair = consts.tile([128, 256], BF16)
        nc.vector.tensor_copy(self.stair[:], stair_st[:])

        # x -> bf16 copy in DRAM (conv1 im2col + xs gather source)
        xst = stage.tile([128, H * W // 128], F32)
        nc.sync.dma_start(
            out=xst[:], in_=_ap(self.x, 0, [(H * W // 128, 128), (1, H * W // 128)])
        )
        xsb = stage.tile([128, H * W // 128], BF16)
        nc.vector.tensor_copy(xsb[:], xst[:])
        nc.sync.dma_start(
            out=_ap(self.xb, 0, [(H * W // 128, 128), (1, H * W // 128)]), in_=xsb[:]
        )
        stage.release()

    # ---------------- per-strip stages ----------------

    def emit_conv1(self, i0, S):
        # Chunk-paired via PE array tiling: chunk a (first half-strip) runs
        # in tile (0,0) [SBUF 0-31 -> PSUM 0-63], chunk b (second half) in
        # tile (64,64) [SBUF 64-95 -> PSUM 64-127], concurrently. relu-a
        # writes h lower; relu-b (lanes 64-127) writes h upper pre-shifted;
        # coarse dup DMAs fill in the opposite halves and build h1b.
        nc = self.nc
        c0 = i0 + 6
        L1 = (S + 6) * W
        L2 = (S + 3) * W
        LB = L2 + 320
        h1t = self.p_h1.tile([2 * CH, (S_STRIP + 6) * W + 772], BF16, tag="h1")
        h1b = self.p_h1b.tile([2 * CH, (S_STRIP + 4) * W + 320], BF16, tag="h1b")
        nc.gpsimd.memset(h1t[0:CH, L1 : L1 + 772], 0.0)
        nc.gpsimd.memset(h1t[CH:, L1 - 1 : L1 + 771], 0.0)
        Lh = (L1 // (2 * NC_)) * NC_
        groups = list(range(0, L1, 2 * NC_))
        imcs = {}

        def emit_dmas():
            for hs in groups:
                he = min(hs + 2 * NC_, L1)
                imc = self.p_imc.tile([9, 2 * NC_], BF16, tag="imc")
                nc.sync.dma_start(
                    out=imc[:, 0 : he - hs],
                    in_=_ap(self.xb, (c0 - 5) * W - 1 + hs,
                            [(W, 3), (1, 3), (1, he - hs)]),
                )
                imcs[hs] = imc

        def emit_groups(grps):
            for hs in grps:
                he = min(hs + 2 * NC_, L1)
                imc = imcs[hs]
                for cs in range(hs, he, NC_):
                    ce = min(cs + NC_, L1)
                    pt = self.ps12.tile([CH, NC_], F32, tag="ps12")
                    _mm(nc, pt[:, 0 : ce - cs], self.w1sb[0:9, :],
                        imc[:, cs - hs : ce - hs], True, True)
                    nc.scalar.activation(
                        h1t[0:CH, cs:ce], pt[:, 0 : ce - cs], AF.Relu,
                        bias=self.b1sb[0:CH],
                    )
                    dup = nc.gpsimd if DUP_GP else nc.sync
                    if ce == Lh:
                        dup.dma_start(out=h1t[CH:, 0 : Lh - 1], in_=h1t[0:CH, 1:Lh])
                        dup.dma_start(out=h1b[0:CH, 0:Lh], in_=h1t[0:CH, 0:Lh])
                        dup.dma_start(out=h1b[CH:, 0 : Lh - W], in_=h1t[0:CH, W:Lh])
                    elif ce == L1:
                        dup.dma_start(
                            out=h1t[CH:, Lh - 1 : L1 - 1], in_=h1t[0:CH, Lh:L1]
                        )
                        dup.dma_start(out=h1b[0:CH, Lh:LB], in_=h1t[0:CH, Lh:LB])
                        dup.dma_start(
                            out=h1b[CH:, Lh - W : LB], in_=h1t[0:CH, Lh : LB + W]
                        )

        # split into three bursts so conv1's scalar-relu chain (684ns vs
        # ~290ns mm) doesn't back up the in-order PE queue in one long run
        return (h1t, h1b, emit_dmas, lambda: emit_groups(groups[:2]),
                lambda: emit_groups(groups[2:4]),
                lambda: emit_groups(groups[4:]))

    def emit_xs(self, i0, S):
        # xs[(u,v), i*W + j] = x[i0+u+i, j+v]: one contiguous read per
        # tap-row u (13 partitions x (S-1)*W+244 elements) into the spaced
        # layout; cols 244..256 of each row hold neighbor-row junk that the
        # stt views never touch. Tap-row u=9 straddles the 128-tap split:
        # taps 117..127 land in xs_lo[117:128], taps 128..129 in xs_hi[0:2].
        nc = self.nc
        LS = (S - 1) * W + HO
        xs_lo = self.p_xsl.tile([CLO, S_STRIP * W], BF16, tag="xsl")
        eng_lo = nc.gpsimd if XS_GP else nc.scalar
        eng_hi = nc.gpsimd if XS_GP else nc.sync
        for u in range(9):
            eng_lo.dma_start(
                out=xs_lo[u * K : (u + 1) * K, 0:LS],
                in_=_ap(self.xb, (i0 + u) * W, [(1, K), (1, LS)]),
            )
        eng_lo.dma_start(
            out=xs_lo[117:128, 0:LS],
            in_=_ap(self.xb, (i0 + 9) * W, [(1, 11), (1, LS)]),
        )
        xs_hi = self.p_xsh.tile([CHI, S_STRIP * W], BF16, tag="xsh")
        eng_hi.dma_start(
            out=xs_hi[0:2, 0:LS],
            in_=_ap(self.xb, (i0 + 9) * W + 11, [(1, 2), (1, LS)]),
        )
        for u in range(10, 13):
            eng_hi.dma_start(
                out=xs_hi[2 + (u - 10) * K : 2 + (u - 9) * K, 0:LS],
                in_=_ap(self.xb, (i0 + u) * W, [(1, K), (1, LS)]),
            )
        return xs_lo, xs_hi

    def emit_conv2(self, i0, S, h1t, h1b):
        # Chunk parity alternates [single K=64 | pairs K=128] and
        # [pairs | single] so same-row-size groups meet across chunk
        # boundaries: one PE row-size-transition drain per chunk.
        # Also builds the conv3 tiles h2t=[h2; h2<<1] / h2b=[h2; h2<<W]
        # via dup DMAs per half-strip.
        nc = self.nc
        L2 = (S + 3) * W
        LB = S * W + 320
        h2t = self.p_h2.tile([2 * CH, (S_STRIP + 3) * W + 772], BF16, tag="h2")
        h2b = self.p_h2b.tile([2 * CH, S_STRIP * W + 320], BF16, tag="h2b")
        nc.gpsimd.memset(h2t[0:CH, L2 : L2 + 772], 0.0)
        nc.gpsimd.memset(h2t[CH:, L2 - 1 : L2 + 771], 0.0)
        Lh = (L2 // (2 * NC_)) * NC_
        for ci, cs in enumerate(range(0, L2, NC_)):
            ce = min(cs + NC_, L2)
            pt = self.ps12.tile([CH, NC_], F32, tag="ps12")

            def pairs(first, stop):
                for p in range(3):
                    off = p * W + 255
                    _mm(nc, pt[:, 0 : ce - cs],
                        self.w2p[:, p * CH : (p + 1) * CH],
                        h1t[:, cs + off : ce + off], first and p == 0, False)
                offb = 2 + 255
                _mm(nc, pt[:, 0 : ce - cs], self.w2pB[:],
                    h1b[:, cs + offb : ce + offb], False, stop)

            def single(first, stop):
                off = 2 * W + 2 + 255
                _mm(nc, pt[:, 0 : ce - cs], self.w2s[:],
                    h1t[0:CH, cs + off : ce + off], first, stop)

            if ci % 2 == 0:
                single(True, False)
                pairs(False, True)
            else:
                pairs(True, False)
                single(False, True)
            nc.scalar.activation(
                h2t[0:CH, cs:ce], pt[:, 0 : ce - cs], AF.Relu, bias=self.b2sb[0:CH]
            )
            dup = nc.gpsimd if DUP_GP else nc.sync
            if ce == Lh:
                dup.dma_start(out=h2t[CH:, 0 : Lh - 1], in_=h2t[0:CH, 1:Lh])
                dup.dma_start(out=h2b[0:CH, 0:Lh], in_=h2t[0:CH, 0:Lh])
                dup.dma_start(out=h2b[CH:, 0 : Lh - W], in_=h2t[0:CH, W:Lh])
            elif ce == L2:
                dup.dma_start(out=h2t[CH:, Lh - 1 : L2 - 1], in_=h2t[0:CH, Lh:L2])
                dup.dma_start(out=h2b[0:CH, Lh:LB], in_=h2t[0:CH, Lh:LB])
                dup.dma_start(
                    out=h2b[CH:, Lh - W : LB], in_=h2t[0:CH, Lh : LB + W]
                )
        return h2t, h2b

    def emit_conv3_chunk(self, i0, cs, h2t, h2b, xs_lo, xs_hi, flush):
        """conv3 + stt for one 2-row chunk; staircase matmuls are deferred.

        Chunk parity alternates [singles K=64 | stair_hi K=41 | stair_lo +
        pairs K=128] and [pairs + stair_lo | singles + stair_hi] so
        consecutive chunks share row sizes at their boundary: one PE drain
        per chunk (K=41 rounds to the 64-row tile config)."""
        nc = self.nc
        even = self.gchunk % 2 == 0
        self.gchunk += 1
        plo = self.ps3.tile([CLO, NC_], F32, tag="ps3lo")
        phi = self.ps3.tile([CHI, NC_], F32, tag="ps3hi")

        def singles(first, stop):
            off = 2 * W + 2 + 255
            rhs = h2t[0:CH, cs + off : cs + NC_ + off]
            _mm(nc, plo[:], self.w3s_lo[:], rhs, first, stop)
            _mm(nc, phi[:], self.w3s_hi[:], rhs, first, stop)

        def pairs(first, stop):
            for p in range(3):
                off = p * W + 255
                rhs = h2t[:, cs + off : cs + NC_ + off]
                _mm(nc, plo[:], self.w3pA_lo[:, p * CLO : (p + 1) * CLO],
                    rhs, first and p == 0, False)
                _mm(nc, phi[:], self.w3pA_hi[:, p * CHI : (p + 1) * CHI],
                    rhs, first and p == 0, False)
            offb = 2 + 255
            rhsb = h2b[:, cs + offb : cs + NC_ + offb]
            _mm(nc, plo[:], self.w3pB_lo[:], rhsb, False, stop)
            _mm(nc, phi[:], self.w3pB_hi[:], rhsb, False, stop)

        flush_hi, flush_lo = flush
        if even:
            singles(True, False)
            flush_hi()
            flush_lo()
            pairs(False, True)
        else:
            pairs(True, False)
            flush_lo()
            singles(False, True)
            flush_hi()

        r2 = cs // W
        jj = (i0 + r2) // 2
        # t2 = (conv3_psum + b3) * xs, straight from PSUM on the DVE; all
        # three operands live in the 256-spaced [c, (r, col)] layout.
        t2 = self.p_t2.tile([CLO, NC_], BF16, tag="t2")
        t_hi = self.p_th.tile([CHI, NC_], BF16, tag="t_hi")
        wv_lo = plo[:].rearrange("p (r c) -> p r c", c=W)[:, :, 6 : 6 + HO]
        wv_hi = phi[:].rearrange("p (r c) -> p r c", c=W)[:, :, 6 : 6 + HO]
        xv_lo = xs_lo[:, cs : cs + NC_].rearrange("p (r c) -> p r c", c=W)[:, :, 0:HO]
        xv_hi = xs_hi[:, cs : cs + NC_].rearrange("p (r c) -> p r c", c=W)[:, :, 0:HO]
        tv_lo = t2[:].rearrange("p (r c) -> p r c", c=W)[:, :, 0:HO]
        tv_hi = t_hi[:].rearrange("p (r c) -> p r c", c=W)[:, :, 0:HO]
        nc.vector.scalar_tensor_tensor(
            out=tv_lo, in0=wv_lo, scalar=self.b3lo[:], in1=xv_lo,
            op0=ALU.add, op1=ALU.mult,
        )
        nc.vector.scalar_tensor_tensor(
            out=tv_hi, in0=wv_hi, scalar=self.b3hi[:], in1=xv_hi,
            op0=ALU.add, op1=ALU.mult,
        )
        self.pend_lo.append((t2, jj))
        self.pend_hi.append((t_hi, jj))

    def _stair_mm(self, t_t, np_, jj):
        # psum_y is split into two [64, 488] banks so the staircase lhsT is
        # only M=64 columns (stationary load halves; only one col is ones).
        # Each 2-row chunk jj lands as TWO stair matmuls: K=128 over t2 and
        # K=41 over t_hi (rounds to the singles' 64-row tile config, so each
        # flush sits transition-free inside its same-row-size region).
        nc = self.nc
        rhs = t_t[0:np_].rearrange("p (r c) -> p r c", c=W)[:, :, 0:HO]
        if jj < 64:
            _mm(nc, self.psum_ya[:], self.stair[0:np_, 128 - jj : 192 - jj],
                rhs, self.cnt_a == 0, self.cnt_a == 2 * 64 - 1)
            self.cnt_a += 1
        else:
            _mm(nc, self.psum_yb[:], self.stair[0:np_, 192 - jj : 256 - jj],
                rhs, self.cnt_b == 0, self.cnt_b == 2 * (self.NYC - 64) - 1)
            self.cnt_b += 1

    def flush_stair_lo(self, keep=0):
        while len(self.pend_lo) > keep:
            t2, jj = self.pend_lo.pop(0)
            self._stair_mm(t2, CLO, jj)

    def flush_stair_hi(self, keep=0):
        while len(self.pend_hi) > keep:
            t_hi, jj = self.pend_hi.pop(0)
            self._stair_mm(t_hi, CHI, jj)

    def _build_strips(self):
        nc = self.nc
        self.NYC = (HO * HO) // 488  # 122
        self.psum_ya = self.psy.tile([64, 488], F32, tag="ya")
        self.psum_yb = self.psy.tile([64, 488], F32, tag="yb")
        self.pend_lo = []
        self.pend_hi = []
        self.cnt_a = 0
        self.cnt_b = 0
        self.gchunk = 0

        strips = []
        i0 = 0
        while i0 < HO:
            strips.append((i0, min(S_STRIP, HO - i0)))
            i0 += S_STRIP

        h1t, h1b, c1d, c1a, c1b, c1c = self.emit_conv1(*strips[0])
        c1d()
        c1a()
        c1b()
        c1c()
        xs = self.emit_xs(*strips[0])
        keep = 4 if PIPE else 0
        flush = (lambda: self.flush_stair_hi(keep=keep),
                 lambda: self.flush_stair_lo(keep=keep))
        for si, (i0, S) in enumerate(strips):
            h2t, h2b = self.emit_conv2(i0, S, h1t, h1b)
            xs_lo, xs_hi = xs
            # prefetch next strip's xs while this strip's conv3 runs
            if si + 1 < len(strips):
                xs = self.emit_xs(*strips[si + 1])
                h1n, h1bn, c1d, c1a, c1b, c1c = self.emit_conv1(*strips[si + 1])
                c1d()
            else:
                c1a = c1b = c1c = None
            for ci, cs in enumerate(range(0, S * W, NC_)):
                self.emit_conv3_chunk(
                    i0, cs, h2t, h2b, xs_lo, xs_hi, flush=flush,
                )
                # overlap next strip's conv1 with this strip's conv3 tail,
                # split into three bursts
                if PIPE and ci == 1 and c1a is not None:
                    c1a()
                    c1a = None
                if PIPE and ci == 3 and c1b is not None:
                    c1b()
                    c1b = None
                if PIPE and ci == 5 and c1c is not None:
                    c1c()
                    c1c = None
            for fn in (c1a, c1b, c1c):
                if fn is not None:
                    fn()
            if si + 1 < len(strips):
                h1t, h1b = h1n, h1bn
            self.flush_stair_hi(keep=0)
            self.flush_stair_lo(keep=0)

        ysba = self.p_y.tile([64, 488], F32, tag="ya")
        nc.vector.tensor_copy(ysba[:], self.psum_ya[:])
        nc.sync.dma_start(out=_ap(self.y, 0, [(488, 64), (1, 488)]), in_=ysba[:])
        nb = self.NYC - 64  # 58
        ysbb = self.p_y.tile([64, 488], F32, tag="ya")
        nc.vector.tensor_copy(ysbb[0:nb, :], self.psum_yb[0:nb, :])
        nc.sync.dma_start(
            out=_ap(self.y, 64 * 488, [(488, nb), (1, 488)]), in_=ysbb[0:nb, :]
        )


_NC_CACHE = {}


def _get_nc():
    if "nc" not in _NC_CACHE:
        _NC_CACHE["nc"] = build_nc()
    return _NC_CACHE["nc"]


def _in_maps(inputs):
    x = np.ascontiguousarray(np.asarray(inputs["x"], dtype=np.float32))
    names = ["W1", "b1", "W2", "b2", "W3", "b3"]
    ws = {n: np.ascontiguousarray(np.asarray(inputs[n], np.float32)) for n in names}
    maps = []
    for i in range(8):
        m = {"x": x[i : i + 1]}
        m.update(ws)
        maps.append(m)
    return maps


def kernel(**inputs):
    nc = _get_nc()
    res = run_bass_kernel_spmd(nc, _in_maps(inputs), list(range(8)))
    return np.concatenate([res.results[i]["y"] for i in range(8)], axis=0)


def profile(**inputs):
    nc = _get_nc()
    res = run_bass_kernel_spmd(nc, _in_maps(inputs), list(range(8)), trace=True)
    return res.exec_time_ns


if __name__ == "__main__":
    rng = np.random.RandomState(0)
    ins = {
        "x": rng.randn(8, 1, H, W).astype(np.float32),
        "W1": rng.randn(CH, 1, 3, 3).astype(np.float32) * 0.1,
        "b1": np.zeros(CH, np.float32),
        "W2": rng.randn(CH, CH, 3, 3).astype(np.float32) * 0.05,
        "b2": np.zeros(CH, np.float32),
        "W3": rng.randn(C3, CH, 3, 3).astype(np.float32) * 0.05,
        "b3": np.zeros(C3, np.float32),
    }
    print(kernel(**ins).shape)


# revision 23
# speedup vs baseline: 1.4650x; 1.1144x over previous
"""NlmCNN (weight-predicting CNN + per-pixel 13x13 weighted sum) on 8 trn2 cores.

Sharding: data-parallel over batch (8 images -> 8 cores), weights replicated.

Per-core layout trick: output y is the conv stack's result center-cropped by
6 pixels, and the receptive field of the three 3x3 convs only reaches 3 px
out, so SAME-padding never materializes: every layer is computed VALID-style
on an unpadded 256-stride flat layout. Column-wrap junk from flat shifted
reads stays confined to the outer <=3 columns of each layer, which are
discarded by the crop.

All matmul operands are bf16 (fp32 PSUM accumulation; end-to-end absmax-rel
~4e-3 vs the 2e-2 gate). bf16 is chosen over float32r because fp32-class
LDWEIGHTS runs ~2.2ns/stationary-column with FWL disabled: M=128 weight
loads (285ns) exceed the N=512 stream time (213ns) and the PE becomes
weight-load-bound (measured 426ns/matmul cadence). bf16 enables FWL and
streams the same 1 column/cycle.

Pipeline per strip of S output rows (strips software-pipelined: conv1 of
strip i+1 is emitted during strip i's conv3 phase):
  conv1: per-2-chunk im2col [9, 1024] via one 3-dim DMA -> K=9 matmul; all
         of a strip's im2col DMAs are issued a strip ahead (the imc pool
         holds a full strip) so the PE never waits on DMA latency.
  conv2/conv3: 3x3 taps packed into K=128 pair-matmuls: taps (du,0)+(du,1)
         pair on hA=[h; h<<1] (upper 64 partitions hold h shifted +1);
         taps (0,2)+(1,2) pair on hB=[h; h<<W]; only tap (2,2) is a K=64
         single. 5 matmuls per 512-px chunk for conv2, 10 for conv3
         (out channels split [0:128] M=128 / [128:169] M=41). The shifted
         tiles are built by SBUF->SBUF dup DMAs per half-strip. Chunk
         parity alternates [single | pairs] / [pairs | single] so the PE
         sees one K-row-size transition per chunk.
  einsum: patch matrix xs[t, s] = x[pos + shift(t)] split [128 | 41] taps,
         gathered bf16 by one contiguous DMA per tap-row u (row u=9 is
         split 11/2 across the two tiles); DVE scalar_tensor_tensor
         computes t2 = (conv3_lo + b3_lo) * xs_lo and
         t_hi = (conv3_hi + b3_hi) * xs_hi straight from PSUM; the
         partition reduction is "staircase ones" matmuls (K=128 over t2
         next to the pairs, K=41 over t_hi next to the K=64 singles --
         both transition-free), accumulating 2-row chunk jj into row jj
         of a persistent PSUM tile; one copy + one DMA store the image.
"""

import numpy as np

import concourse.bacc as bacc
import concourse.bass as bass
import concourse.mybir as mybir
import concourse.tile as tile
from concourse.bass_utils import run_bass_kernel_spmd

F32 = mybir.dt.float32
BF16 = mybir.dt.bfloat16
AF = mybir.ActivationFunctionType
ALU = mybir.AluOpType

H = 256
W = 256
K = 13
HO = H - K + 1  # 244
CH = 64
C3 = K * K  # 169
CLO = 128   # conv3 out-channel group sizes
CHI = C3 - 128  # 41
S_STRIP = 16
NC_ = 512  # chunk positions (2 image rows)
import os
PIPE = os.environ.get("K_PIPE", "1") == "1"      # cross-strip sw pipelining
XS_GP = os.environ.get("K_XS_GP", "1") == "1"    # xs DMAs on gpsimd queue
DUP_GP = os.environ.get("K_DUP_GP", "1") == "1"  # dup DMAs on gpsimd queue


def _ap(t, off, dims):
    return bass.AP(t, off, [list(d) for d in dims])


def _mm(nc, out, lhsT, rhs, start, stop):
    nc.tensor.matmul(out, lhsT, rhs, start=start, stop=stop)


def build_nc():
    nc = bacc.Bacc("TRN2", target_bir_lowering=False, debug=False)

    x = nc.dram_tensor("x", [1, 1, H, W], F32, kind="ExternalInput")
    w1 = nc.dram_tensor("W1", [CH, 1, 3, 3], F32, kind="ExternalInput")
    b1 = nc.dram_tensor("b1", [CH], F32, kind="ExternalInput")
    w2 = nc.dram_tensor("W2", [CH, CH, 3, 3], F32, kind="ExternalInput")
    b2 = nc.dram_tensor("b2", [CH], F32, kind="ExternalInput")
    w3 = nc.dram_tensor("W3", [C3, CH, 3, 3], F32, kind="ExternalInput")
    b3 = nc.dram_tensor("b3", [C3], F32, kind="ExternalInput")
    y = nc.dram_tensor("y", [1, 1, HO, HO], F32, kind="ExternalOutput")
    xb = nc.dram_tensor("x_b", [H * W], BF16)

    with tile.TileContext(nc) as tc:
        Body(nc, tc, x, w1, b1, w2, b2, w3, b3, y, xb).build()

    nc.compile()
    return nc


class Body:
    def __init__(self, nc, tc, x, w1, b1, w2, b2, w3, b3, y, xb):
        self.nc, self.tc = nc, tc
        self.x, self.w1, self.b1, self.w2, self.b2 = x, w1, b1, w2, b2
        self.w3, self.b3, self.y, self.xb = w3, b3, y, xb

    def build(self):
        nc, tc = self.nc, self.tc
        with (
            tc.tile_pool(name="consts", bufs=1) as consts,
            tc.tile_pool(name="t2p", bufs=5) as p_t2,
            tc.tile_pool(name="thp", bufs=5) as p_th,
            tc.tile_pool(name="imc", bufs=6) as p_imc,
            tc.tile_pool(name="h1p", bufs=2) as p_h1,
            tc.tile_pool(name="h1bp", bufs=2) as p_h1b,
            tc.tile_pool(name="h2p", bufs=2) as p_h2,
            tc.tile_pool(name="h2bp", bufs=2) as p_h2b,
            tc.tile_pool(name="xsl", bufs=2) as p_xsl,
            tc.tile_pool(name="xsh", bufs=2) as p_xsh,
            tc.tile_pool(name="yout", bufs=1) as p_y,
            tc.tile_pool(name="ps12", bufs=3, space="PSUM") as ps12,
            tc.tile_pool(name="ps3", bufs=2, space="PSUM") as ps3,
            tc.tile_pool(name="psy", bufs=1, space="PSUM") as psy,
        ):
            self.consts = consts
            self.p_t2, self.p_th, self.p_imc = p_t2, p_th, p_imc
            self.p_h1, self.p_h1b = p_h1, p_h1b
            self.p_h2, self.p_h2b = p_h2, p_h2b
            self.p_xsl, self.p_xsh = p_xsl, p_xsh
            self.p_y, self.ps12, self.ps3, self.psy = p_y, ps12, ps3, psy
            self._build_consts()
            self._build_strips()

    def _build_consts(self):
        nc, tc, consts = self.nc, self.tc, self.consts
        stage = tc.alloc_tile_pool(name="stage", bufs=1)
        # weight-prep transposes borrow ps3's "ps3lo" slot (same max tile
        # size, consts-time only) so no dedicated PSUM bank is needed
        pwtr = self.ps3

        # Weights arrive [co, ci, du, dv]; matmuls need [ci, co] per tap.
        # A strided gather DMA would be 4-byte-descriptor-bound, so load
        # contiguously and transpose on the PE instead.
        from concourse.masks import make_identity

        ident = stage.tile([128, 128], F32)
        make_identity(nc, ident[:])

        w1raw = stage.tile([CH, 9], F32)
        nc.sync.dma_start(out=w1raw[:], in_=_ap(self.w1, 0, [(9, CH), (1, 9)]))
        w2raw = stage.tile([CH, 9 * CH], F32)
        nc.sync.dma_start(out=w2raw[:], in_=_ap(self.w2, 0, [(9 * CH, CH), (1, 9 * CH)]))
        w3raw_a = stage.tile([128, 9 * CH], F32)
        nc.sync.dma_start(
            out=w3raw_a[:], in_=_ap(self.w3, 0, [(9 * CH, 128), (1, 9 * CH)])
        )
        w3raw_b = stage.tile([CHI, 9 * CH], F32)
        nc.sync.dma_start(
            out=w3raw_b[:],
            in_=_ap(self.w3, 128 * 9 * CH, [(9 * CH, CHI), (1, 9 * CH)]),
        )

        def tapv(raw, t, n):  # [n_co, ci] view of tap t
            return raw[0:n, :].rearrange("p (ci t) -> p t ci", t=9)[:, t, :]

        # w1: lhsT [9 taps, 64 co]; copy at partitions 64-72 for the
        # row-tiled chunk-b matmul (lhs/rhs start partitions must match)
        pT = pwtr.tile([128, 128], F32, tag="ps3lo")
        nc.tensor.transpose(pT[0:9, 0:CH], w1raw[:], ident[0:CH, 0:CH])
        self.w1sb = consts.tile([128, CH], BF16)
        nc.vector.tensor_copy(self.w1sb[0:9, :], pT[0:9, 0:CH])
        nc.sync.dma_start(out=self.w1sb[64:73, :], in_=self.w1sb[0:9, :])

        # Transpose each tap to PSUM base 0 (HW requires base 0); upper
        # (shifted-partner tap) halves staged then partition-shifted to
        # partitions 64-127 by one SBUF->SBUF DMA per weight tile.
        # A-pairs carry taps (du,0)+(du,1) du=0..2; B-pair (0,2)+(1,2);
        # single is tap (2,2); conv3 splits co into [0:128] / [128:169].
        self.w2p = consts.tile([2 * CH, 3 * CH], BF16)
        self.w2pB = consts.tile([2 * CH, CH], BF16)
        self.w2s = consts.tile([CH, CH], BF16)
        self.w3pA_lo = consts.tile([2 * CH, 3 * CLO], BF16)
        self.w3pA_hi = consts.tile([2 * CH, 3 * CHI], BF16)
        self.w3pB_lo = consts.tile([2 * CH, CLO], BF16)
        self.w3pB_hi = consts.tile([2 * CH, CHI], BF16)
        self.w3s_lo = consts.tile([CH, CLO], BF16)
        self.w3s_hi = consts.tile([CH, CHI], BF16)
        w2pu = stage.tile([CH, 3 * CH], BF16)
        w2puB = stage.tile([CH, CH], BF16)
        w3puA_lo = stage.tile([CH, 3 * CLO], BF16)
        w3puA_hi = stage.tile([CH, 3 * CHI], BF16)
        w3puB_lo = stage.tile([CH, CLO], BF16)
        w3puB_hi = stage.tile([CH, CHI], BF16)

        def tr(dst, raw, t, n):
            pT = pwtr.tile([CH, 128], F32, tag="ps3lo")
            nc.tensor.transpose(pT[:, 0:n], tapv(raw, t, n), ident[0:n, 0:n])
            nc.vector.tensor_copy(dst, pT[:, 0:n])

        for p in range(3):
            cw = slice(p * CH, (p + 1) * CH)
            cl = slice(p * CLO, (p + 1) * CLO)
            ch = slice(p * CHI, (p + 1) * CHI)
            tr(self.w2p[0:CH, cw], w2raw, p * 3, CH)
            tr(w2pu[:, cw], w2raw, p * 3 + 1, CH)
            tr(self.w3pA_lo[0:CH, cl], w3raw_a, p * 3, 128)
            tr(self.w3pA_hi[0:CH, ch], w3raw_b, p * 3, CHI)
            tr(w3puA_lo[:, cl], w3raw_a, p * 3 + 1, 128)
            tr(w3puA_hi[:, ch], w3raw_b, p * 3 + 1, CHI)
        tr(self.w2pB[0:CH, :], w2raw, 2, CH)
        tr(w2puB[:], w2raw, 5, CH)
        tr(self.w2s[:], w2raw, 8, CH)
        tr(self.w3pB_lo[0:CH, :], w3raw_a, 2, 128)
        tr(self.w3pB_hi[0:CH, :], w3raw_b, 2, CHI)
        tr(w3puB_lo[:], w3raw_a, 5, 128)
        tr(w3puB_hi[:], w3raw_b, 5, CHI)
        tr(self.w3s_lo[:], w3raw_a, 8, 128)
        tr(self.w3s_hi[:], w3raw_b, 8, CHI)
        nc.sync.dma_start(out=self.w2p[CH:, :], in_=w2pu[:])
        nc.sync.dma_start(out=self.w2pB[CH:, :], in_=w2puB[:])
        nc.sync.dma_start(out=self.w3pA_lo[CH:, :], in_=w3puA_lo[:])
        nc.sync.dma_start(out=self.w3pA_hi[CH:, :], in_=w3puA_hi[:])
        nc.sync.dma_start(out=self.w3pB_lo[CH:, :], in_=w3puB_lo[:])
        nc.sync.dma_start(out=self.w3pB_hi[CH:, :], in_=w3puB_hi[:])

        # biases replicated into partitions 64-127 for the chunk-b relus
        # (engine lanes are partition-hardwired)
        self.b1sb = consts.tile([2 * CH, 1], F32)
        nc.scalar.dma_start(out=self.b1sb[0:CH], in_=_ap(self.b1, 0, [(1, CH), (0, 1)]))
        nc.scalar.dma_start(out=self.b1sb[CH:], in_=_ap(self.b1, 0, [(1, CH), (0, 1)]))
        self.b2sb = consts.tile([2 * CH, 1], F32)
        nc.scalar.dma_start(out=self.b2sb[0:CH], in_=_ap(self.b2, 0, [(1, CH), (0, 1)]))
        nc.scalar.dma_start(out=self.b2sb[CH:], in_=_ap(self.b2, 0, [(1, CH), (0, 1)]))
        self.b3lo = consts.tile([CLO, 1], F32)
        nc.scalar.dma_start(out=self.b3lo[:], in_=_ap(self.b3, 0, [(1, CLO), (0, 1)]))
        self.b3hi = consts.tile([CHI, 1], F32)
        nc.scalar.dma_start(out=self.b3hi[:], in_=_ap(self.b3, CLO, [(1, CHI), (0, 1)]))

        # staircase-ones: stair[:, 128] = 1, else 0; column j of the view
        # stair[:, 128-j : 192-j] is all-ones -> matmul writes the partition
        # sum into PSUM row j (zeros elsewhere, harmless under accumulation)
        stair_st = stage.tile([128, 256], F32)
        nc.vector.memset(stair_st[:], 0.0)
        nc.vector.memset(stair_st[:, 128:129], 1.0)
        self.stair = consts.tile([128, 256], BF16)
        nc.vector.tensor_copy(self.stair[:], stair_st[:])

        # x -> bf16 copy in DRAM (conv1 im2col + xs gather source)
        xst = stage.tile([128, H * W // 128], F32)
        nc.sync.dma_start(
            out=xst[:], in_=_ap(self.x, 0, [(H * W // 128, 128), (1, H * W // 128)])
        )
        xsb = stage.tile([128, H * W // 128], BF16)
        nc.vector.tensor_copy(xsb[:], xst[:])
        nc.sync.dma_start(
            out=_ap(self.xb, 0, [(H * W // 128, 128), (1, H * W // 128)]), in_=xsb[:]
        )
        stage.release()

    # ---------------- per-strip stages ----------------

    def emit_conv1(self, i0, S):
        # Chunk-paired via PE array tiling: chunk a (first half-strip) runs
        # in tile (0,0) [SBUF 0-31 -> PSUM 0-63], chunk b (second half) in
        # tile (64,64) [SBUF 64-95 -> PSUM 64-127], concurrently. relu-a
        # writes h lower; relu-b (lanes 64-127) writes h upper pre-shifted;
        # coarse dup DMAs fill in the opposite halves and build h1b.
        nc = self.nc
        c0 = i0 + 6
        L1 = (S + 6) * W
        L2 = (S + 3) * W
        LB = L2 + 320
        h1t = self.p_h1.tile([2 * CH, (S_STRIP + 6) * W + 772], BF16, tag="h1")
        h1b = self.p_h1b.tile([2 * CH, (S_STRIP + 4) * W + 320], BF16, tag="h1b")
        nc.gpsimd.memset(h1t[0:CH, L1 : L1 + 772], 0.0)
        nc.gpsimd.memset(h1t[CH:, L1 - 1 : L1 + 771], 0.0)
        Lh = (L1 // (2 * NC_)) * NC_
        groups = list(range(0, L1, 2 * NC_))
        imcs = {}

        def emit_dmas():
            for hs in groups:
                he = min(hs + 2 * NC_, L1)
                imc = self.p_imc.tile([9, 2 * NC_], BF16, tag="imc")
                nc.sync.dma_start(
                    out=imc[:, 0 : he - hs],
                    in_=_ap(self.xb, (c0 - 5) * W - 1 + hs,
                            [(W, 3), (1, 3), (1, he - hs)]),
                )
                imcs[hs] = imc

        def emit_groups(grps):
            for hs in grps:
                he = min(hs + 2 * NC_, L1)
                imc = imcs[hs]
                for cs in range(hs, he, NC_):
                    ce = min(cs + NC_, L1)
                    pt = self.ps12.tile([CH, NC_], F32, tag="ps12")
                    _mm(nc, pt[:, 0 : ce - cs], self.w1sb[0:9, :],
                        imc[:, cs - hs : ce - hs], True, True)
                    nc.scalar.activation(
                        h1t[0:CH, cs:ce], pt[:, 0 : ce - cs], AF.Relu,
                        bias=self.b1sb[0:CH],
                    )
                    dup = nc.gpsimd if DUP_GP else nc.sync
                    if ce == Lh:
                        dup.dma_start(out=h1t[CH:, 0 : Lh - 1], in_=h1t[0:CH, 1:Lh])
                        dup.dma_start(out=h1b[0:CH, 0:Lh], in_=h1t[0:CH, 0:Lh])
                        dup.dma_start(out=h1b[CH:, 0 : Lh - W], in_=h1t[0:CH, W:Lh])
                    elif ce == L1:
                        dup.dma_start(
                            out=h1t[CH:, Lh - 1 : L1 - 1], in_=h1t[0:CH, Lh:L1]
                        )
                        dup.dma_start(out=h1b[0:CH, Lh:LB], in_=h1t[0:CH, Lh:LB])
                        dup.dma_start(
                            out=h1b[CH:, Lh - W : LB], in_=h1t[0:CH, Lh : LB + W]
                        )

        # split into three bursts so conv1's scalar-relu chain (684ns vs
        # ~290ns mm) doesn't back up the in-order PE queue in one long run
        return (h1t, h1b, emit_dmas, lambda: emit_groups(groups[:2]),
                lambda: emit_groups(groups[2:4]),
                lambda: emit_groups(groups[4:]))

    def emit_xs(self, i0, S):
        # xs[(u,v), i*W + j] = x[i0+u+i, j+v]: one contiguous read per
        # tap-row u (13 partitions x (S-1)*W+244 elements) into the spaced
        # layout; cols 244..256 of each row hold neighbor-row junk that the
        # stt views never touch. Tap-row u=9 straddles the 128-tap split:
        # taps 117..127 land in xs_lo[117:128], taps 128..129 in xs_hi[0:2].
        nc = self.nc
        LS = (S - 1) * W + HO
        xs_lo = self.p_xsl.tile([CLO, S_STRIP * W], BF16, tag="xsl")
        eng_lo = nc.gpsimd if XS_GP else nc.scalar
        eng_hi = nc.gpsimd if XS_GP else nc.sync
        for u in range(9):
            eng_lo.dma_start(
                out=xs_lo[u * K : (u + 1) * K, 0:LS],
                in_=_ap(self.xb, (i0 + u) * W, [(1, K), (1, LS)]),
            )
        eng_lo.dma_start(
            out=xs_lo[117:128, 0:LS],
            in_=_ap(self.xb, (i0 + 9) * W, [(1, 11), (1, LS)]),
        )
        xs_hi = self.p_xsh.tile([CHI, S_STRIP * W], BF16, tag="xsh")
        eng_hi.dma_start(
            out=xs_hi[0:2, 0:LS],
            in_=_ap(self.xb, (i0 + 9) * W + 11, [(1, 2), (1, LS)]),
        )
        for u in range(10, 13):
            eng_hi.dma_start(
                out=xs_hi[2 + (u - 10) * K : 2 + (u - 9) * K, 0:LS],
                in_=_ap(self.xb, (i0 + u) * W, [(1, K), (1, LS)]),
            )
        return xs_lo, xs_hi

    def emit_conv2(self, i0, S, h1t, h1b):
        # Chunk parity alternates [single K=64 | pairs K=128] and
        # [pairs | single] so same-row-size groups meet across chunk
        # boundaries: one PE row-size-transition drain per chunk.
        # Also builds the conv3 tiles h2t=[h2; h2<<1] / h2b=[h2; h2<<W]
        # via dup DMAs per half-strip.
        nc = self.nc
        L2 = (S + 3) * W
        LB = S * W + 320
        h2t = self.p_h2.tile([2 * CH, (S_STRIP + 3) * W + 772], BF16, tag="h2")
        h2b = self.p_h2b.tile([2 * CH, S_STRIP * W + 320], BF16, tag="h2b")
        nc.gpsimd.memset(h2t[0:CH, L2 : L2 + 772], 0.0)
        nc.gpsimd.memset(h2t[CH:, L2 - 1 : L2 + 771], 0.0)
        Lh = (L2 // (2 * NC_)) * NC_
        for ci, cs in enumerate(range(0, L2, NC_)):
            ce = min(cs + NC_, L2)
            pt = self.ps12.tile([CH, NC_], F32, tag="ps12")

            def pairs(first, stop):
                for p in range(3):
                    off = p * W + 255
                    _mm(nc, pt[:, 0 : ce - cs],
                        self.w2p[:, p * CH : (p + 1) * CH],
                        h1t[:, cs + off : ce + off], first and p == 0, False)
                offb = 2 + 255
                _mm(nc, pt[:, 0 : ce - cs], self.w2pB[:],
                    h1b[:, cs + offb : ce + offb], False, stop)

            def single(first, stop):
                off = 2 * W + 2 + 255
                _mm(nc, pt[:, 0 : ce - cs], self.w2s[:],
                    h1t[0:CH, cs + off : ce + off], first, stop)

            if ci % 2 == 0:
                single(True, False)
                pairs(False, True)
            else:
                pairs(True, False)
                single(False, True)
            nc.scalar.activation(
                h2t[0:CH, cs:ce], pt[:, 0 : ce - cs], AF.Relu, bias=self.b2sb[0:CH]
            )
            dup = nc.gpsimd if DUP_GP else nc.sync
            if ce == Lh:
                dup.dma_start(out=h2t[CH:, 0 : Lh - 1], in_=h2t[0:CH, 1:Lh])
                dup.dma_start(out=h2b[0:CH, 0:Lh], in_=h2t[0:CH, 0:Lh])
                dup.dma_start(out=h2b[CH:, 0 : Lh - W], in_=h2t[0:CH, W:Lh])
            elif ce == L2:
                dup.dma_start(out=h2t[CH:, Lh - 1 : L2 - 1], in_=h2t[0:CH, Lh:L2])
                dup.dma_start(out=h2b[0:CH, Lh:LB], in_=h2t[0:CH, Lh:LB])
                dup.dma_start(
                    out=h2b[CH:, Lh - W : LB], in_=h2t[0:CH, Lh : LB + W]
                )
        return h2t, h2b

    def emit_conv3_chunk(self, i0, cs, h2t, h2b, xs_lo, xs_hi, flush):
        """conv3 + stt for one 2-row chunk; staircase matmuls are deferred.

        The PE pays ~100ns whenever consecutive matmuls change stationary
        geometry (row OR column size), so matmuls are grouped by geometry:
        all M=128 (lo) work together, then all M=41 (hi). The M=128 stair
        (over t2) is geometry-identical to the lo pairs and the K=41 stair
        (over t_hi; 41 rounds to the 64-row config) matches the K=64
        singles class. Chunk parity mirrors the order so chunk boundaries
        are transition-free: ~3 geometry changes per chunk total."""
        nc = self.nc
        even = self.gchunk % 2 == 0
        self.gchunk += 1
        plo = self.ps3.tile([CLO, NC_], F32, tag="ps3lo")
        phi = self.ps3.tile([CHI, NC_], F32, tag="ps3hi")

        def single_lo(first, stop):
            off = 2 * W + 2 + 255
            _mm(nc, plo[:], self.w3s_lo[:],
                h2t[0:CH, cs + off : cs + NC_ + off], first, stop)

        def single_hi(first, stop):
            off = 2 * W + 2 + 255
            _mm(nc, phi[:], self.w3s_hi[:],
                h2t[0:CH, cs + off : cs + NC_ + off], first, stop)

        def pairs_lo(first, stop):
            for p in range(3):
                off = p * W + 255
                _mm(nc, plo[:], self.w3pA_lo[:, p * CLO : (p + 1) * CLO],
                    h2t[:, cs + off : cs + NC_ + off], first and p == 0, False)
            offb = 2 + 255
            _mm(nc, plo[:], self.w3pB_lo[:],
                h2b[:, cs + offb : cs + NC_ + offb], False, stop)

        def pairs_hi(first, stop):
            for p in range(3):
                off = p * W + 255
                _mm(nc, phi[:], self.w3pA_hi[:, p * CHI : (p + 1) * CHI],
                    h2t[:, cs + off : cs + NC_ + off], first and p == 0, False)
            offb = 2 + 255
            _mm(nc, phi[:], self.w3pB_hi[:],
                h2b[:, cs + offb : cs + NC_ + offb], False, stop)

        flush_hi, flush_lo = flush
        if even:
            flush_hi()
            single_lo(True, False)
            pairs_lo(False, True)
            flush_lo()
            pairs_hi(True, False)
            single_hi(False, True)
        else:
            single_hi(True, False)
            pairs_hi(False, True)
            flush_lo()
            pairs_lo(True, False)
            single_lo(False, True)
            flush_hi()

        r2 = cs // W
        jj = (i0 + r2) // 2
        # t2 = (conv3_psum + b3) * xs, straight from PSUM on the DVE; all
        # three operands live in the 256-spaced [c, (r, col)] layout.
        t2 = self.p_t2.tile([CLO, NC_], BF16, tag="t2")
        t_hi = self.p_th.tile([CHI, NC_], BF16, tag="t_hi")
        wv_lo = plo[:].rearrange("p (r c) -> p r c", c=W)[:, :, 6 : 6 + HO]
        wv_hi = phi[:].rearrange("p (r c) -> p r c", c=W)[:, :, 6 : 6 + HO]
        xv_lo = xs_lo[:, cs : cs + NC_].rearrange("p (r c) -> p r c", c=W)[:, :, 0:HO]
        xv_hi = xs_hi[:, cs : cs + NC_].rearrange("p (r c) -> p r c", c=W)[:, :, 0:HO]
        tv_lo = t2[:].rearrange("p (r c) -> p r c", c=W)[:, :, 0:HO]
        tv_hi = t_hi[:].rearrange("p (r c) -> p r c", c=W)[:, :, 0:HO]
        nc.vector.scalar_tensor_tensor(
            out=tv_lo, in0=wv_lo, scalar=self.b3lo[:], in1=xv_lo,
            op0=ALU.add, op1=ALU.mult,
        )
        nc.vector.scalar_tensor_tensor(
            out=tv_hi, in0=wv_hi, scalar=self.b3hi[:], in1=xv_hi,
            op0=ALU.add, op1=ALU.mult,
        )
        self.pend_lo.append((t2, jj))
        self.pend_hi.append((t_hi, jj))

    def _stair_mm(self, t_t, np_, jj):
        # psum_y is one [128, 488] bank accumulating all 122 output chunks;
        # the staircase lhsT is M=128 so the lo stair shares the lo pairs'
        # stationary geometry exactly (no PE reconfiguration drain).
        nc = self.nc
        rhs = t_t[0:np_].rearrange("p (r c) -> p r c", c=W)[:, :, 0:HO]
        _mm(nc, self.psum_y[:], self.stair[0:np_, 128 - jj : 256 - jj],
            rhs, self.cnt == 0, self.cnt == 2 * self.NYC - 1)
        self.cnt += 1

    def flush_stair_lo(self, keep=0):
        while len(self.pend_lo) > keep:
            t2, jj = self.pend_lo.pop(0)
            self._stair_mm(t2, CLO, jj)

    def flush_stair_hi(self, keep=0):
        while len(self.pend_hi) > keep:
            t_hi, jj = self.pend_hi.pop(0)
            self._stair_mm(t_hi, CHI, jj)

    def _build_strips(self):
        nc = self.nc
        self.NYC = (HO * HO) // 488  # 122
        self.psum_y = self.psy.tile([128, 488], F32, tag="y")
        self.pend_lo = []
        self.pend_hi = []
        self.cnt = 0
        self.gchunk = 0

        strips = []
        i0 = 0
        while i0 < HO:
            strips.append((i0, min(S_STRIP, HO - i0)))
            i0 += S_STRIP

        h1t, h1b, c1d, c1a, c1b, c1c = self.emit_conv1(*strips[0])
        c1d()
        c1a()
        c1b()
        c1c()
        xs = self.emit_xs(*strips[0])
        keep = 4 if PIPE else 0
        flush = (lambda: self.flush_stair_hi(keep=keep),
                 lambda: self.flush_stair_lo(keep=keep))
        for si, (i0, S) in enumerate(strips):
            h2t, h2b = self.emit_conv2(i0, S, h1t, h1b)
            xs_lo, xs_hi = xs
            # prefetch next strip's xs while this strip's conv3 runs
            if si + 1 < len(strips):
                xs = self.emit_xs(*strips[si + 1])
                h1n, h1bn, c1d, c1a, c1b, c1c = self.emit_conv1(*strips[si + 1])
                c1d()
            else:
                c1a = c1b = c1c = None
            for ci, cs in enumerate(range(0, S * W, NC_)):
                self.emit_conv3_chunk(
                    i0, cs, h2t, h2b, xs_lo, xs_hi, flush=flush,
                )
                # overlap next strip's conv1 with this strip's conv3 tail,
                # split into three bursts
                if PIPE and ci == 1 and c1a is not None:
                    c1a()
                    c1a = None
                if PIPE and ci == 3 and c1b is not None:
                    c1b()
                    c1b = None
                if PIPE and ci == 5 and c1c is not None:
                    c1c()
                    c1c = None
            for fn in (c1a, c1b, c1c):
                if fn is not None:
                    fn()
            if si + 1 < len(strips):
                h1t, h1b = h1n, h1bn
            self.flush_stair_hi(keep=0)
            self.flush_stair_lo(keep=0)

        ysb = self.p_y.tile([self.NYC, 488], F32, tag="y")
        nc.vector.tensor_copy(ysb[:], self.psum_y[0 : self.NYC, :])
        nc.sync.dma_start(
            out=_ap(self.y, 0, [(488, self.NYC), (1, 488)]), in_=ysb[:]
        )


_NC_CACHE = {}


def _get_nc():
    if "nc" not in _NC_CACHE:
        _NC_CACHE["nc"] = build_nc()
    return _NC_CACHE["nc"]


def _in_maps(inputs):
    x = np.ascontiguousarray(np.asarray(inputs["x"], dtype=np.float32))
    names = ["W1", "b1", "W2", "b2", "W3", "b3"]
    ws = {n: np.ascontiguousarray(np.asarray(inputs[n], np.float32)) for n in names}
    maps = []
    for i in range(8):
        m = {"x": x[i : i + 1]}
        m.update(ws)
        maps.append(m)
    return maps


def kernel(**inputs):
    nc = _get_nc()
    res = run_bass_kernel_spmd(nc, _in_maps(inputs), list(range(8)))
    return np.concatenate([res.results[i]["y"] for i in range(8)], axis=0)


def profile(**inputs):
    nc = _get_nc()
    res = run_bass_kernel_spmd(nc, _in_maps(inputs), list(range(8)), trace=True)
    return res.exec_time_ns


if __name__ == "__main__":
    rng = np.random.RandomState(0)
    ins = {
        "x": rng.randn(8, 1, H, W).astype(np.float32),
        "W1": rng.randn(CH, 1, 3, 3).astype(np.float32) * 0.1,
        "b1": np.zeros(CH, np.float32),
        "W2": rng.randn(CH, CH, 3, 3).astype(np.float32) * 0.05,
        "b2": np.zeros(CH, np.float32),
        "W3": rng.randn(C3, CH, 3, 3).astype(np.float32) * 0.05,
        "b3": np.zeros(C3, np.float32),
    }
    print(kernel(**ins).shape)


# revision 37
# speedup vs baseline: 1.6380x; 1.1181x over previous
"""NlmCNN (weight-predicting CNN + per-pixel 13x13 weighted sum) on 8 trn2 cores.

Sharding: data-parallel over batch (8 images -> 8 cores), weights replicated.

Per-core layout trick: output y is the conv stack's result center-cropped by
6 pixels, and the receptive field of the three 3x3 convs only reaches 3 px
out, so SAME-padding never materializes: every layer is computed VALID-style
on an unpadded 256-stride flat layout. Column-wrap junk from flat shifted
reads stays confined to the outer <=3 columns of each layer, which are
discarded by the crop.

All matmul operands are bf16 (fp32 PSUM accumulation; end-to-end absmax-rel
~4e-3 vs the 2e-2 gate). bf16 is chosen over float32r because fp32-class
LDWEIGHTS runs ~2.2ns/stationary-column with FWL disabled: M=128 weight
loads (285ns) exceed the N=512 stream time (213ns) and the PE becomes
weight-load-bound (measured 426ns/matmul cadence). bf16 enables FWL and
streams the same 1 column/cycle.

Pipeline per strip of S output rows (strips software-pipelined: conv1 of
strip i+1 is emitted during strip i's conv3 phase):
  conv1: per-2-chunk im2col [9, 1024] via one 3-dim DMA -> K=9 matmul; all
         of a strip's im2col DMAs are issued a strip ahead (the imc pool
         holds a full strip) so the PE never waits on DMA latency.
  conv2/conv3: 3x3 taps packed into K=128 pair-matmuls: taps (du,0)+(du,1)
         pair on hA=[h; h<<1] (upper 64 partitions hold h shifted +1);
         taps (0,2)+(1,2) pair on hB=[h; h<<W]; only tap (2,2) is a K=64
         single. 5 matmuls per 512-px chunk for conv2, 10 for conv3
         (out channels split [0:128] M=128 / [128:169] M=41). The shifted
         tiles are built by SBUF->SBUF dup DMAs per half-strip. Chunk
         parity alternates [single | pairs] / [pairs | single] so the PE
         sees one K-row-size transition per chunk.
  einsum: patch matrix xs[t, s] = x[pos + shift(t)] split [128 | 41] taps,
         gathered bf16 by one contiguous DMA per tap-row u (row u=9 is
         split 11/2 across the two tiles); DVE scalar_tensor_tensor
         computes t2 = (conv3_lo + b3_lo) * xs_lo and
         t_hi = (conv3_hi + b3_hi) * xs_hi straight from PSUM; the
         partition reduction is "staircase ones" matmuls (K=128 over t2
         next to the pairs, K=41 over t_hi next to the K=64 singles --
         both transition-free), accumulating 2-row chunk jj into row jj
         of a persistent PSUM tile; one copy + one DMA store the image.
"""

import numpy as np

import concourse.bacc as bacc
import concourse.bass as bass
import concourse.mybir as mybir
import concourse.tile as tile
from concourse.bass_utils import run_bass_kernel_spmd

F32 = mybir.dt.float32
BF16 = mybir.dt.bfloat16
AF = mybir.ActivationFunctionType
ALU = mybir.AluOpType

H = 256
W = 256
K = 13
HO = H - K + 1  # 244
CH = 64
C3 = K * K  # 169
CLO = 128   # conv3 out-channel group sizes
CHI = C3 - 128  # 41
S_STRIP = 16
NC_ = 512  # chunk positions (2 image rows)
import os
PIPE = os.environ.get("K_PIPE", "1") == "1"      # cross-strip sw pipelining
XS_GP = os.environ.get("K_XS_GP", "1") == "1"    # xs DMAs on gpsimd queue
DUP_GP = os.environ.get("K_DUP_GP", "1") == "1"  # dup DMAs on gpsimd queue


def _ap(t, off, dims):
    return bass.AP(t, off, [list(d) for d in dims])


def _mm(nc, out, lhsT, rhs, start, stop):
    nc.tensor.matmul(out, lhsT, rhs, start=start, stop=stop)


def build_nc():
    nc = bacc.Bacc("TRN2", target_bir_lowering=False, debug=False)

    x = nc.dram_tensor("x", [1, 1, H, W], F32, kind="ExternalInput")
    w1 = nc.dram_tensor("W1", [CH, 1, 3, 3], F32, kind="ExternalInput")
    b1 = nc.dram_tensor("b1", [CH], F32, kind="ExternalInput")
    w2 = nc.dram_tensor("W2", [CH, CH, 3, 3], F32, kind="ExternalInput")
    b2 = nc.dram_tensor("b2", [CH], F32, kind="ExternalInput")
    w3 = nc.dram_tensor("W3", [C3, CH, 3, 3], F32, kind="ExternalInput")
    b3 = nc.dram_tensor("b3", [C3], F32, kind="ExternalInput")
    y = nc.dram_tensor("y", [1, 1, HO, HO], F32, kind="ExternalOutput")
    xb = nc.dram_tensor("x_b", [H * W], BF16)

    with tile.TileContext(nc) as tc:
        Body(nc, tc, x, w1, b1, w2, b2, w3, b3, y, xb).build()

    nc.compile()
    return nc


class Body:
    def __init__(self, nc, tc, x, w1, b1, w2, b2, w3, b3, y, xb):
        self.nc, self.tc = nc, tc
        self.x, self.w1, self.b1, self.w2, self.b2 = x, w1, b1, w2, b2
        self.w3, self.b3, self.y, self.xb = w3, b3, y, xb

    def build(self):
        nc, tc = self.nc, self.tc
        with (
            tc.tile_pool(name="consts", bufs=1) as consts,
            tc.tile_pool(name="t2p", bufs=5) as p_t2,
            tc.tile_pool(name="thp", bufs=5) as p_th,
            tc.tile_pool(name="imc", bufs=6) as p_imc,
            tc.tile_pool(name="h1p", bufs=2) as p_h1,
            tc.tile_pool(name="h1bp", bufs=2) as p_h1b,
            tc.tile_pool(name="h2p", bufs=2) as p_h2,
            tc.tile_pool(name="h2bp", bufs=2) as p_h2b,
            tc.tile_pool(name="xsl", bufs=2) as p_xsl,
            tc.tile_pool(name="xsh", bufs=2) as p_xsh,
            tc.tile_pool(name="yout", bufs=1) as p_y,
            tc.tile_pool(name="ps12", bufs=3, space="PSUM") as ps12,
            tc.tile_pool(name="ps3", bufs=2, space="PSUM") as ps3,
            tc.tile_pool(name="psy", bufs=1, space="PSUM") as psy,
        ):
            self.consts = consts
            self.p_t2, self.p_th, self.p_imc = p_t2, p_th, p_imc
            self.p_h1, self.p_h1b = p_h1, p_h1b
            self.p_h2, self.p_h2b = p_h2, p_h2b
            self.p_xsl, self.p_xsh = p_xsl, p_xsh
            self.p_y, self.ps12, self.ps3, self.psy = p_y, ps12, ps3, psy
            self._build_consts()
            self._build_strips()

    def _build_consts(self):
        nc, tc, consts = self.nc, self.tc, self.consts
        stage = tc.alloc_tile_pool(name="stage", bufs=1)
        # weight-prep transposes borrow ps3's "ps3lo" slot (same max tile
        # size, consts-time only) so no dedicated PSUM bank is needed
        pwtr = self.ps3

        # Weights arrive [co, ci, du, dv]; matmuls need [ci, co] per tap.
        # A strided gather DMA would be 4-byte-descriptor-bound, so load
        # contiguously and transpose on the PE instead.
        from concourse.masks import make_identity

        ident = stage.tile([128, 128], F32)
        make_identity(nc, ident[:])

        w1raw = stage.tile([CH, 9], F32)
        nc.sync.dma_start(out=w1raw[:], in_=_ap(self.w1, 0, [(9, CH), (1, 9)]))
        w2raw = stage.tile([CH, 9 * CH], F32)
        nc.sync.dma_start(out=w2raw[:], in_=_ap(self.w2, 0, [(9 * CH, CH), (1, 9 * CH)]))
        w3raw_a = stage.tile([128, 9 * CH], F32)
        nc.sync.dma_start(
            out=w3raw_a[:], in_=_ap(self.w3, 0, [(9 * CH, 128), (1, 9 * CH)])
        )
        w3raw_b = stage.tile([CHI, 9 * CH], F32)
        nc.sync.dma_start(
            out=w3raw_b[:],
            in_=_ap(self.w3, 128 * 9 * CH, [(9 * CH, CHI), (1, 9 * CH)]),
        )

        def tapv(raw, t, n):  # [n_co, ci] view of tap t
            return raw[0:n, :].rearrange("p (ci t) -> p t ci", t=9)[:, t, :]

        # All lhsT tiles are K=128-padded with ZERO rows so every matmul
        # shares the 128-row stationary config: the PE pays ~100ns whenever
        # consecutive matmuls change stationary geometry (row or column
        # count), and a K=128 matmul streams the same N columns as a K=9
        # one. Zero weight rows turn the junk in the corresponding rhs
        # partitions into exact zeros.
        # w1: lhsT rows 0-8 = taps, rows 9-127 = 0.
        pT = pwtr.tile([128, 128], F32, tag="ps3lo")
        nc.tensor.transpose(pT[0:9, 0:CH], w1raw[:], ident[0:CH, 0:CH])
        self.w1sb = consts.tile([128, CH], BF16)
        nc.vector.memset(self.w1sb[:], 0.0)
        nc.vector.tensor_copy(self.w1sb[0:9, :], pT[0:9, 0:CH])

        # Transpose each tap to PSUM base 0 (HW requires base 0); upper
        # (shifted-partner tap) halves staged then partition-shifted to
        # partitions 64-127 by one SBUF->SBUF DMA per weight tile.
        # A-pairs carry taps (du,0)+(du,1) du=0..2; B-pair (0,2)+(1,2);
        # single is tap (2,2); conv3 splits co into [0:128] / [128:169].
        self.w2p = consts.tile([2 * CH, 3 * CH], BF16)
        self.w2pB = consts.tile([2 * CH, CH], BF16)
        self.w2s = consts.tile([2 * CH, CH], BF16)
        self.w3pA_lo = consts.tile([2 * CH, 3 * CLO], BF16)
        self.w3pA_hi = consts.tile([2 * CH, 3 * CHI], BF16)
        self.w3pB_lo = consts.tile([2 * CH, CLO], BF16)
        self.w3pB_hi = consts.tile([2 * CH, CHI], BF16)
        self.w3s_lo = consts.tile([2 * CH, CLO], BF16)
        self.w3s_hi = consts.tile([2 * CH, CHI], BF16)
        nc.vector.memset(self.w2s[CH:, :], 0.0)
        nc.vector.memset(self.w3s_lo[CH:, :], 0.0)
        nc.vector.memset(self.w3s_hi[CH:, :], 0.0)
        w2pu = stage.tile([CH, 3 * CH], BF16)
        w2puB = stage.tile([CH, CH], BF16)
        w3puA_lo = stage.tile([CH, 3 * CLO], BF16)
        w3puA_hi = stage.tile([CH, 3 * CHI], BF16)
        w3puB_lo = stage.tile([CH, CLO], BF16)
        w3puB_hi = stage.tile([CH, CHI], BF16)

        def tr(dst, raw, t, n):
            pT = pwtr.tile([CH, 128], F32, tag="ps3lo")
            nc.tensor.transpose(pT[:, 0:n], tapv(raw, t, n), ident[0:n, 0:n])
            nc.vector.tensor_copy(dst, pT[:, 0:n])

        for p in range(3):
            cw = slice(p * CH, (p + 1) * CH)
            cl = slice(p * CLO, (p + 1) * CLO)
            ch = slice(p * CHI, (p + 1) * CHI)
            tr(self.w2p[0:CH, cw], w2raw, p * 3, CH)
            tr(w2pu[:, cw], w2raw, p * 3 + 1, CH)
            tr(self.w3pA_lo[0:CH, cl], w3raw_a, p * 3, 128)
            tr(self.w3pA_hi[0:CH, ch], w3raw_b, p * 3, CHI)
            tr(w3puA_lo[:, cl], w3raw_a, p * 3 + 1, 128)
            tr(w3puA_hi[:, ch], w3raw_b, p * 3 + 1, CHI)
        tr(self.w2pB[0:CH, :], w2raw, 2, CH)
        tr(w2puB[:], w2raw, 5, CH)
        tr(self.w2s[0:CH, :], w2raw, 8, CH)
        tr(self.w3pB_lo[0:CH, :], w3raw_a, 2, 128)
        tr(self.w3pB_hi[0:CH, :], w3raw_b, 2, CHI)
        tr(w3puB_lo[:], w3raw_a, 5, 128)
        tr(w3puB_hi[:], w3raw_b, 5, CHI)
        tr(self.w3s_lo[0:CH, :], w3raw_a, 8, 128)
        tr(self.w3s_hi[0:CH, :], w3raw_b, 8, CHI)
        nc.sync.dma_start(out=self.w2p[CH:, :], in_=w2pu[:])
        nc.sync.dma_start(out=self.w2pB[CH:, :], in_=w2puB[:])
        nc.sync.dma_start(out=self.w3pA_lo[CH:, :], in_=w3puA_lo[:])
        nc.sync.dma_start(out=self.w3pA_hi[CH:, :], in_=w3puA_hi[:])
        nc.sync.dma_start(out=self.w3pB_lo[CH:, :], in_=w3puB_lo[:])
        nc.sync.dma_start(out=self.w3pB_hi[CH:, :], in_=w3puB_hi[:])

        # biases replicated into partitions 64-127 for the chunk-b relus
        # (engine lanes are partition-hardwired)
        self.b1sb = consts.tile([2 * CH, 1], F32)
        nc.scalar.dma_start(out=self.b1sb[0:CH], in_=_ap(self.b1, 0, [(1, CH), (0, 1)]))
        nc.scalar.dma_start(out=self.b1sb[CH:], in_=_ap(self.b1, 0, [(1, CH), (0, 1)]))
        self.b2sb = consts.tile([2 * CH, 1], F32)
        nc.scalar.dma_start(out=self.b2sb[0:CH], in_=_ap(self.b2, 0, [(1, CH), (0, 1)]))
        nc.scalar.dma_start(out=self.b2sb[CH:], in_=_ap(self.b2, 0, [(1, CH), (0, 1)]))
        self.b3lo = consts.tile([CLO, 1], F32)
        nc.scalar.dma_start(out=self.b3lo[:], in_=_ap(self.b3, 0, [(1, CLO), (0, 1)]))
        self.b3hi = consts.tile([CHI, 1], F32)
        nc.scalar.dma_start(out=self.b3hi[:], in_=_ap(self.b3, CLO, [(1, CHI), (0, 1)]))

        # staircase-ones: stair[:, 128] = 1, else 0; column j of the view
        # stair[:, 128-j : 192-j] is all-ones -> matmul writes the partition
        # sum into PSUM row j (zeros elsewhere, harmless under accumulation)
        stair_st = stage.tile([128, 256], F32)
        nc.vector.memset(stair_st[:], 0.0)
        nc.vector.memset(stair_st[:, 128:129], 1.0)
        self.stair = consts.tile([128, 256], BF16)
        nc.vector.tensor_copy(self.stair[:], stair_st[:])
        # hi-stair variant: ones only in rows 0..40 (t_hi's live taps) so a
        # K=128 matmul over the zero-padded t_hi reduces exactly 41 rows
        self.stair_h = consts.tile([128, 256], BF16)
        nc.vector.memset(self.stair_h[:], 0.0)
        nc.vector.tensor_copy(self.stair_h[0:CHI, :], stair_st[0:CHI, :])

        # x -> bf16 copy in DRAM (conv1 im2col + xs gather source)
        xst = stage.tile([128, H * W // 128], F32)
        nc.sync.dma_start(
            out=xst[:], in_=_ap(self.x, 0, [(H * W // 128, 128), (1, H * W // 128)])
        )
        xsb = stage.tile([128, H * W // 128], BF16)
        nc.vector.tensor_copy(xsb[:], xst[:])
        nc.sync.dma_start(
            out=_ap(self.xb, 0, [(H * W // 128, 128), (1, H * W // 128)]), in_=xsb[:]
        )
        stage.release()

    # ---------------- per-strip stages ----------------

    def emit_conv1(self, i0, S, first_strip=False):
        # im2col DMAs for the whole strip are issued up-front (the 6-deep
        # imc pool holds a full strip) so the PE never waits on DMA latency
        # when the deferred matmul bursts run a strip later. imc tiles are
        # K=128-padded: rows 9..127 are zeroed once (first strip touches
        # every pool slot) and w1sb's zero rows make them inert.
        nc = self.nc
        c0 = i0 + 6
        L1 = (S + 6) * W
        L2 = (S + 3) * W
        LB = L2 + 320
        h1t = self.p_h1.tile([2 * CH, (S_STRIP + 6) * W + 772], BF16, tag="h1")
        h1b = self.p_h1b.tile([2 * CH, (S_STRIP + 4) * W + 320], BF16, tag="h1b")
        nc.gpsimd.memset(h1t[0:CH, L1 : L1 + 772], 0.0)
        nc.gpsimd.memset(h1t[CH:, L1 - 1 : L1 + 771], 0.0)
        Lh = (L1 // (2 * NC_)) * NC_
        groups = list(range(0, L1, 2 * NC_))
        imcs = {}

        def emit_dmas():
            for hs in groups:
                he = min(hs + 2 * NC_, L1)
                imc = self.p_imc.tile([128, 2 * NC_], BF16, tag="imc")
                if first_strip:
                    nc.gpsimd.memset(imc[:], 0.0)
                nc.sync.dma_start(
                    out=imc[0:9, 0 : he - hs],
                    in_=_ap(self.xb, (c0 - 5) * W - 1 + hs,
                            [(W, 3), (1, 3), (1, he - hs)]),
                )
                imcs[hs] = imc

        def emit_groups(grps):
            for hs in grps:
                he = min(hs + 2 * NC_, L1)
                imc = imcs[hs]
                for cs in range(hs, he, NC_):
                    ce = min(cs + NC_, L1)
                    pt = self.ps12.tile([CH, NC_], F32, tag="ps12")
                    _mm(nc, pt[:, 0 : ce - cs], self.w1sb[:],
                        imc[:, cs - hs : ce - hs], True, True)
                    nc.scalar.activation(
                        h1t[0:CH, cs:ce], pt[:, 0 : ce - cs], AF.Relu,
                        bias=self.b1sb[0:CH],
                    )
                    dup = nc.gpsimd if DUP_GP else nc.sync
                    if ce == Lh:
                        dup.dma_start(out=h1t[CH:, 0 : Lh - 1], in_=h1t[0:CH, 1:Lh])
                        dup.dma_start(out=h1b[0:CH, 0:Lh], in_=h1t[0:CH, 0:Lh])
                        dup.dma_start(out=h1b[CH:, 0 : Lh - W], in_=h1t[0:CH, W:Lh])
                    elif ce == L1:
                        dup.dma_start(
                            out=h1t[CH:, Lh - 1 : L1 - 1], in_=h1t[0:CH, Lh:L1]
                        )
                        dup.dma_start(out=h1b[0:CH, Lh:LB], in_=h1t[0:CH, Lh:LB])
                        dup.dma_start(
                            out=h1b[CH:, Lh - W : LB], in_=h1t[0:CH, Lh : LB + W]
                        )

        # split into three bursts so conv1's scalar-relu chain (684ns vs
        # ~290ns mm) doesn't back up the in-order PE queue in one long run
        return (h1t, h1b, emit_dmas, lambda: emit_groups(groups[:2]),
                lambda: emit_groups(groups[2:4]),
                lambda: emit_groups(groups[4:]))

    def emit_xs(self, i0, S):
        # xs[(u,v), i*W + j] = x[i0+u+i, j+v]: one contiguous read per
        # tap-row u (13 partitions x (S-1)*W+244 elements) into the spaced
        # layout; cols 244..256 of each row hold neighbor-row junk that the
        # stt views never touch. Tap-row u=9 straddles the 128-tap split:
        # taps 117..127 land in xs_lo[117:128], taps 128..129 in xs_hi[0:2].
        nc = self.nc
        LS = (S - 1) * W + HO
        xs_lo = self.p_xsl.tile([CLO, S_STRIP * W], BF16, tag="xsl")
        eng_lo = nc.gpsimd if XS_GP else nc.scalar
        eng_hi = nc.gpsimd if XS_GP else nc.sync
        for u in range(9):
            eng_lo.dma_start(
                out=xs_lo[u * K : (u + 1) * K, 0:LS],
                in_=_ap(self.xb, (i0 + u) * W, [(1, K), (1, LS)]),
            )
        eng_lo.dma_start(
            out=xs_lo[117:128, 0:LS],
            in_=_ap(self.xb, (i0 + 9) * W, [(1, 11), (1, LS)]),
        )
        xs_hi = self.p_xsh.tile([CHI, S_STRIP * W], BF16, tag="xsh")
        eng_hi.dma_start(
            out=xs_hi[0:2, 0:LS],
            in_=_ap(self.xb, (i0 + 9) * W + 11, [(1, 2), (1, LS)]),
        )
        for u in range(10, 13):
            eng_hi.dma_start(
                out=xs_hi[2 + (u - 10) * K : 2 + (u - 9) * K, 0:LS],
                in_=_ap(self.xb, (i0 + u) * W, [(1, K), (1, LS)]),
            )
        return xs_lo, xs_hi

    def emit_conv2(self, i0, S, h1t, h1b):
        # Chunk parity alternates [single K=64 | pairs K=128] and
        # [pairs | single] so same-row-size groups meet across chunk
        # boundaries: one PE row-size-transition drain per chunk.
        # Also builds the conv3 tiles h2t=[h2; h2<<1] / h2b=[h2; h2<<W]
        # via dup DMAs per half-strip.
        nc = self.nc
        L2 = (S + 3) * W
        LB = S * W + 320
        h2t = self.p_h2.tile([2 * CH, (S_STRIP + 3) * W + 772], BF16, tag="h2")
        h2b = self.p_h2b.tile([2 * CH, S_STRIP * W + 320], BF16, tag="h2b")
        nc.gpsimd.memset(h2t[0:CH, L2 : L2 + 772], 0.0)
        nc.gpsimd.memset(h2t[CH:, L2 - 1 : L2 + 771], 0.0)
        Lh = (L2 // (2 * NC_)) * NC_
        for ci, cs in enumerate(range(0, L2, NC_)):
            ce = min(cs + NC_, L2)
            pt = self.ps12.tile([CH, NC_], F32, tag="ps12")

            for p in range(3):
                off = p * W + 255
                _mm(nc, pt[:, 0 : ce - cs],
                    self.w2p[:, p * CH : (p + 1) * CH],
                    h1t[:, cs + off : ce + off], p == 0, False)
            offb = 2 + 255
            _mm(nc, pt[:, 0 : ce - cs], self.w2pB[:],
                h1b[:, cs + offb : ce + offb], False, False)
            offs = 2 * W + 2 + 255
            _mm(nc, pt[:, 0 : ce - cs], self.w2s[:],
                h1t[:, cs + offs : ce + offs], False, True)
            nc.scalar.activation(
                h2t[0:CH, cs:ce], pt[:, 0 : ce - cs], AF.Relu, bias=self.b2sb[0:CH]
            )
            dup = nc.gpsimd if DUP_GP else nc.sync
            if ce == Lh:
                dup.dma_start(out=h2t[CH:, 0 : Lh - 1], in_=h2t[0:CH, 1:Lh])
                dup.dma_start(out=h2b[0:CH, 0:Lh], in_=h2t[0:CH, 0:Lh])
                dup.dma_start(out=h2b[CH:, 0 : Lh - W], in_=h2t[0:CH, W:Lh])
            elif ce == L2:
                dup.dma_start(out=h2t[CH:, Lh - 1 : L2 - 1], in_=h2t[0:CH, Lh:L2])
                dup.dma_start(out=h2b[0:CH, Lh:LB], in_=h2t[0:CH, Lh:LB])
                dup.dma_start(
                    out=h2b[CH:, Lh - W : LB], in_=h2t[0:CH, Lh : LB + W]
                )
        return h2t, h2b

    def emit_conv3_chunk(self, i0, cs, h2t, h2b, xs_lo, xs_hi, flush):
        """conv3 + stt for one 2-row chunk; staircase matmuls are deferred.

        Every matmul is K=128 (zero-padded weights for the lone single
        tap), so the only stationary-geometry changes are M: the M=128
        block (lo pairs+single, both stairs) and the M=41 block (hi
        pairs+single). Chunk parity mirrors the block order so chunk
        boundaries are transition-free: ONE geometry change per chunk."""
        nc = self.nc
        even = self.gchunk % 2 == 0
        self.gchunk += 1
        plo = self.ps3.tile([CLO, NC_], F32, tag="ps3lo")
        phi = self.ps3.tile([CHI, NC_], F32, tag="ps3hi")

        def block_lo():
            off = 2 * W + 2 + 255
            _mm(nc, plo[:], self.w3s_lo[:],
                h2t[:, cs + off : cs + NC_ + off], True, False)
            for p in range(3):
                off = p * W + 255
                _mm(nc, plo[:], self.w3pA_lo[:, p * CLO : (p + 1) * CLO],
                    h2t[:, cs + off : cs + NC_ + off], False, False)
            offb = 2 + 255
            _mm(nc, plo[:], self.w3pB_lo[:],
                h2b[:, cs + offb : cs + NC_ + offb], False, True)

        def block_hi():
            off = 2 * W + 2 + 255
            _mm(nc, phi[:], self.w3s_hi[:],
                h2t[:, cs + off : cs + NC_ + off], True, False)
            for p in range(3):
                off = p * W + 255
                _mm(nc, phi[:], self.w3pA_hi[:, p * CHI : (p + 1) * CHI],
                    h2t[:, cs + off : cs + NC_ + off], False, False)
            offb = 2 + 255
            _mm(nc, phi[:], self.w3pB_hi[:],
                h2b[:, cs + offb : cs + NC_ + offb], False, True)

        flush_hi, flush_lo = flush
        if even:
            block_lo()
            flush_lo()
            flush_hi()
            block_hi()
        else:
            block_hi()
            flush_lo()
            flush_hi()
            block_lo()

        r2 = cs // W
        jj = (i0 + r2) // 2
        # t2 = (conv3_psum + b3) * xs, straight from PSUM on the DVE; all
        # three operands live in the 256-spaced [c, (r, col)] layout.
        # t_hi is K=128-padded: rows 41..127 are zeroed once per pool slot
        # (the first 5 chunks touch all 5 slots) and stair_h's zero rows
        # make them inert.
        t2 = self.p_t2.tile([CLO, NC_], BF16, tag="t2")
        t_hi = self.p_th.tile([128, NC_], BF16, tag="t_hi")
        if self.gchunk <= 5:
            nc.gpsimd.memset(t_hi[64:128, :], 0.0)
            nc.gpsimd.memset(t_hi[32:64, :], 0.0)
        wv_lo = plo[:].rearrange("p (r c) -> p r c", c=W)[:, :, 6 : 6 + HO]
        wv_hi = phi[:].rearrange("p (r c) -> p r c", c=W)[:, :, 6 : 6 + HO]
        xv_lo = xs_lo[:, cs : cs + NC_].rearrange("p (r c) -> p r c", c=W)[:, :, 0:HO]
        xv_hi = xs_hi[:, cs : cs + NC_].rearrange("p (r c) -> p r c", c=W)[:, :, 0:HO]
        tv_lo = t2[:].rearrange("p (r c) -> p r c", c=W)[:, :, 0:HO]
        tv_hi = t_hi[0:CHI].rearrange("p (r c) -> p r c", c=W)[:, :, 0:HO]
        nc.vector.scalar_tensor_tensor(
            out=tv_lo, in0=wv_lo, scalar=self.b3lo[:], in1=xv_lo,
            op0=ALU.add, op1=ALU.mult,
        )
        nc.vector.scalar_tensor_tensor(
            out=tv_hi, in0=wv_hi, scalar=self.b3hi[:], in1=xv_hi,
            op0=ALU.add, op1=ALU.mult,
        )
        self.pend_lo.append((t2, jj))
        self.pend_hi.append((t_hi, jj))

    def _stair_mm(self, t_t, stair, jj):
        # psum_y is one [128, 488] bank accumulating all 122 output chunks;
        # the staircase lhsT is K=128 x M=128 so both stairs share the lo
        # pairs' stationary geometry exactly (no PE reconfiguration drain).
        nc = self.nc
        rhs = t_t[:].rearrange("p (r c) -> p r c", c=W)[:, :, 0:HO]
        _mm(nc, self.psum_y[:], stair[:, 128 - jj : 256 - jj],
            rhs, self.cnt == 0, self.cnt == 2 * self.NYC - 1)
        self.cnt += 1

    def flush_stair_lo(self, keep=0):
        while len(self.pend_lo) > keep:
            t2, jj = self.pend_lo.pop(0)
            self._stair_mm(t2, self.stair, jj)

    def flush_stair_hi(self, keep=0):
        while len(self.pend_hi) > keep:
            t_hi, jj = self.pend_hi.pop(0)
            self._stair_mm(t_hi, self.stair_h, jj)

    def _build_strips(self):
        nc = self.nc
        self.NYC = (HO * HO) // 488  # 122
        self.psum_y = self.psy.tile([128, 488], F32, tag="y")
        self.pend_lo = []
        self.pend_hi = []
        self.cnt = 0
        self.gchunk = 0

        strips = []
        i0 = 0
        while i0 < HO:
            strips.append((i0, min(S_STRIP, HO - i0)))
            i0 += S_STRIP

        h1t, h1b, c1d, c1a, c1b, c1c = self.emit_conv1(*strips[0],
                                                       first_strip=True)
        c1d()
        c1a()
        c1b()
        c1c()
        xs = self.emit_xs(*strips[0])
        keep = 4 if PIPE else 0
        flush = (lambda: self.flush_stair_hi(keep=keep),
                 lambda: self.flush_stair_lo(keep=keep))
        for si, (i0, S) in enumerate(strips):
            h2t, h2b = self.emit_conv2(i0, S, h1t, h1b)
            xs_lo, xs_hi = xs
            # prefetch next strip's xs while this strip's conv3 runs
            if si + 1 < len(strips):
                xs = self.emit_xs(*strips[si + 1])
                h1n, h1bn, c1d, c1a, c1b, c1c = self.emit_conv1(*strips[si + 1])
                c1d()
            else:
                c1a = c1b = c1c = None
            for ci, cs in enumerate(range(0, S * W, NC_)):
                self.emit_conv3_chunk(
                    i0, cs, h2t, h2b, xs_lo, xs_hi, flush=flush,
                )
                # overlap next strip's conv1 with this strip's conv3 tail,
                # split into three bursts
                if PIPE and ci == 1 and c1a is not None:
                    c1a()
                    c1a = None
                if PIPE and ci == 3 and c1b is not None:
                    c1b()
                    c1b = None
                if PIPE and ci == 5 and c1c is not None:
                    c1c()
                    c1c = None
            for fn in (c1a, c1b, c1c):
                if fn is not None:
                    fn()
            if si + 1 < len(strips):
                h1t, h1b = h1n, h1bn
            self.flush_stair_hi(keep=0)
            self.flush_stair_lo(keep=0)

        ysb = self.p_y.tile([self.NYC, 488], F32, tag="y")
        nc.vector.tensor_copy(ysb[:], self.psum_y[0 : self.NYC, :])
        nc.sync.dma_start(
            out=_ap(self.y, 0, [(488, self.NYC), (1, 488)]), in_=ysb[:]
        )


_NC_CACHE = {}


def _get_nc():
    if "nc" not in _NC_CACHE:
        _NC_CACHE["nc"] = build_nc()
    return _NC_CACHE["nc"]


def _in_maps(inputs):
    x = np.ascontiguousarray(np.asarray(inputs["x"], dtype=np.float32))
    names = ["W1", "b1", "W2", "b2", "W3", "b3"]
    ws = {n: np.ascontiguousarray(np.asarray(inputs[n], np.float32)) for n in names}
    maps = []
    for i in range(8):
        m = {"x": x[i : i + 1]}
        m.update(ws)
        maps.append(m)
    return maps


def kernel(**inputs):
    nc = _get_nc()
    res = run_bass_kernel_spmd(nc, _in_maps(inputs), list(range(8)))
    return np.concatenate([res.results[i]["y"] for i in range(8)], axis=0)


def profile(**inputs):
    nc = _get_nc()
    res = run_bass_kernel_spmd(nc, _in_maps(inputs), list(range(8)), trace=True)
    return res.exec_time_ns


if __name__ == "__main__":
    rng = np.random.RandomState(0)
    ins = {
        "x": rng.randn(8, 1, H, W).astype(np.float32),
        "W1": rng.randn(CH, 1, 3, 3).astype(np.float32) * 0.1,
        "b1": np.zeros(CH, np.float32),
        "W2": rng.randn(CH, CH, 3, 3).astype(np.float32) * 0.05,
        "b2": np.zeros(CH, np.float32),
        "W3": rng.randn(C3, CH, 3, 3).astype(np.float32) * 0.05,
        "b3": np.zeros(C3, np.float32),
    }
    print(kernel(**ins).shape)
